# revision 43
# baseline (speedup 1.0000x reference)
"""Trainium2 Bass kernel for nn_Block_3616362463321 (dense transformer block).

B=8, T=1024, C=1024, H=16, Dh=64. Data-parallel over batch: core b gets x[b].
Weights replicated to all 8 cores; no collectives.

v2 design (vs baseline):
  - All six weight GEMMs (QKV / proj / fc1 / fc2) run fp8e4m3 with
    perf_mode=DoubleRow: K=256 contraction per pass, 0.5 cycles/row.
    Weights are host-quantized (x1024, clip +-240) into an interleaved
    [q, p, islot, m] layout; activations are quantized on the fly into
    "mega" SBUF tiles [128, 8*1024] fp8 whose (k=c-chunk, t) layout serves
    both the DoubleRow moving-operand view [p, 2, t] and the stationary
    view [p, 2, 128].
  - LayerNorm statistics AND normalization happen in natural [t, c] layout
    (per-partition mean/rstd -> one tensor_scalar), with gamma folded into
    the weights host-side and beta folded into per-output bias columns.
    No DRAM stat bounces.  Normalized activations are PE-transposed as fp8
    (1 cycle/row), 4 blocks batched per PSUM bank.
  - Attention keeps the baseline S^T orientation (fp8 Q/K, bf16 E,
    fp8 V with fused 0.125-ones column for softmax denominators), with:
      * S row-tile pairs (K=64 at rows 0/64) writing one 2-bank PSUM pair
        tile -> a single paired exp per (m, tn, i) on ACT,
      * exact causal trims everywhere (bf16/fp8 matmuls have no N>=256
        restriction),
      * causal masking as a post-exp bf16 triangle multiply (2x DVE mode),
      * reciprocal_approx_fast for denominators + SBUF->SBUF broadcast DMA,
  - Residual/dequant fused into single scalar_tensor_tensor ops; SBUF-only
    elementwise work (x+bias rows) offloaded to the idle GpSimd engine.
  - Bulk DMA spread over the SP / Pool / ACT hardware queues.
"""
import sys

sys.path.insert(0, "/opt/trn_rl_repo")

from contextlib import ExitStack, nullcontext

import numpy as np
import ml_dtypes

import concourse.bacc as bacc
import concourse.bass as bass
import concourse.mybir as mybir
import concourse.tile as tile
from concourse.bass_utils import run_bass_kernel_spmd

P = 128
B, T, C, H = 8, 1024, 1024, 16
Dh = C // H            # 64
EPS = 1e-5
NF = 512               # matmul moving free dim (fp32 PSUM bank limit)
KC = C // P            # 8 c-chunks of 128
QC = C // 256          # 4 c-chunks of 256 (DoubleRow)
TJ = T // P            # 8 t-chunks of 128
TN = T // NF           # 2 t-chunks of 512
F32 = mybir.dt.float32
F32R = mybir.dt.float32r
BF16 = mybir.dt.bfloat16
F8 = mybir.dt.float8e4
ALU = mybir.AluOpType
ACTF = mybir.ActivationFunctionType
DR = mybir.MatmulPerfMode.DoubleRow

WS = 1024.0            # host weight upscale (fp8 range use)
QS = 8.0               # Q/K storage scale
SEXP = (Dh ** -0.5) / (QS * QS)   # exp scale absorbing Q/K storage scales
VIS = 1.0              # V ones-column value (bf16 attnT: true scale)
HS = 8.0               # hT storage scale

N_CORES = 8

_CACHE = {}

F8NP = ml_dtypes.float8_e4m3
BF16NP = ml_dtypes.bfloat16


def _bcast_row_ap(handle_ap, parts):
    """AP reading a [N]-shaped DRAM tensor broadcast across `parts` partitions."""
    return bass.AP(
        tensor=handle_ap.tensor,
        offset=handle_ap.offset,
        ap=[[0, parts], *handle_ap.ap],
    )


def build_nc(loop=1, hwloop=0, phases=7):
    nc = bacc.Bacc("TRN2", target_bir_lowering=False, debug=False)

    x_d = nc.dram_tensor("x", [T, C], F32, kind="ExternalInput")
    w_d = {}
    for nm in ("wq", "wk", "wv", "w1"):
        w_d[nm] = nc.dram_tensor(nm, [QC * P, 2 * C], F8, kind="ExternalInput")
    wp_d = nc.dram_tensor("wp", [C, C], BF16, kind="ExternalInput")
    w2_d = nc.dram_tensor("w2", [C, C], BF16, kind="ExternalInput")
    # bias columns pre-shaped host-side to [P, KC] (contiguous DMA)
    colq_d = nc.dram_tensor("colq", [P, KC], F32, kind="ExternalInput")
    colk_d = nc.dram_tensor("colk", [P, KC], F32, kind="ExternalInput")
    b1c_d = nc.dram_tensor("b1c", [P, KC], F32, kind="ExternalInput")
    bvr_d = nc.dram_tensor("bvr", [C], BF16, kind="ExternalInput")
    bp_d = nc.dram_tensor("bp", [C], F32, kind="ExternalInput")
    b2_d = nc.dram_tensor("b2", [C], F32, kind="ExternalInput")
    out_d = nc.dram_tensor("out", [T, C], F32, kind="ExternalOutput")

    identb_c = nc.inline_tensor(np.eye(P).astype(BF16NP), name="identb_c")
    # post-exp causal keep-mask for a diagonal [s,t] block: keep where s <= t
    tri_np = (np.arange(P)[:, None] <= np.arange(P)[None, :]).astype(BF16NP)
    tri_c = nc.inline_tensor(tri_np, name="tri_c")

    with tile.TileContext(nc) as tc, ExitStack() as ES:
        singles = ES.enter_context(tc.tile_pool(name="singles", bufs=1))
        dram = ES.enter_context(tc.tile_pool(name="drsc", bufs=1, space="DRAM"))

        identb = singles.tile([P, P], BF16)
        nc.sync.dma_start(out=identb[:], in_=identb_c.ap())
        tri01 = singles.tile([P, P], BF16)
        nc.sync.dma_start(out=tri01[:], in_=tri_c.ap())
        epsc = singles.tile([P, 1], F32)
        nc.vector.memset(epsc[:], EPS)
        onesb = singles.tile([1, P], BF16)
        nc.vector.memset(onesb[:], 1.0)

        # bias columns [P, KC]: element (p, k) = vec[k*P + p]
        cols = {}

        def load_col(nm, hd):
            t_ = singles.tile([P, KC], F32, tag=f"col_{nm}", name=f"col_{nm}")
            nc.gpsimd.dma_start(out=t_[:], in_=hd[:, :])
            cols[nm] = t_

        load_col("q", colq_d)
        load_col("k", colk_d)
        load_col("b1", b1c_d)
        bvr = singles.tile([1, C], BF16)
        nc.gpsimd.dma_start(out=bvr[:], in_=bvr_d.ap()[None, :])
        bpb = singles.tile([P, C], F32)
        nc.gpsimd.dma_start(out=bpb[:], in_=_bcast_row_ap(bp_d.ap(), P))
        b2b = singles.tile([P, C], F32)
        nc.gpsimd.dma_start(out=b2b[:], in_=_bcast_row_ap(b2_d.ap(), P))

        # ---- SBUF arena ----
        arena = ES.enter_context(tc.tile_pool(name="arena", bufs=1))

        def mega(tag):
            return arena.tile([P, KC * T], F8, tag=tag, name=tag)

        # weight tiles: [P, 2, C] fp8 per 256-chunk
        def wtiles(nm, share=None):
            tg = share or nm
            return [arena.tile([P, 2, C], F8, tag=f"{tg}_{q}", name=f"{nm}{q}")
                    for q in range(QC)]

        # ---- PSUM pool: declare SP pair tags first (2 banks each), then PA ----
        psum = ES.enter_context(tc.tile_pool(name="psum", bufs=1, space="PSUM"))
        _sp = [0]
        _pa = [0]

        def sptile(shape, dtype, nm="sp"):
            t = psum.tile(list(shape), dtype, tag=f"SP{_sp[0] % 2}",
                          name=f"{nm}{_sp[0]}")
            _sp[0] += 1
            return t

        def patile(shape=(P, NF), nm="pa"):
            t = psum.tile(list(shape), F32, tag=f"PA{_pa[0] % 4}",
                          name=f"{nm}{_pa[0]}")
            _pa[0] += 1
            return t

        # force tag declaration order: SP0, SP1 as [P, 2, NF] f32 (2 banks each)
        _ = psum.tile([P, 2, NF], F32, tag="SP0", name="spdecl0")
        _ = psum.tile([P, 2, NF], F32, tag="SP1", name="spdecl1")

        # weights on the Pool/ACT queues; x and out rows keep SP (+ACT) free-ish
        _dq = [0]
        _dqe = [nc.gpsimd, nc.scalar]

        def bulk_dma(out, in_):
            eng = _dqe[_dq[0] % len(_dqe)]
            _dq[0] += 1
            eng.dma_start(out=out, in_=in_)

        def row_dma(out, in_):
            nc.sync.dma_start(out=out, in_=in_)

        def ln_pass(src_tiles_or_loader, xn_tag, dst_mega, ph, out_rows_dtype=F8):
            """Natural-layout LN: per 128-row chunk j, bn_stats -> mean/rstd
            columns -> one tensor_scalar into an fp8 row tile -> PE-transpose
            (fp8, 4 blocks per PSUM batch) into dst_mega [(k t)] layout."""
            with ExitStack() as S:
                stp = S.enter_context(tc.tile_pool(name=f"stp{ph}", bufs=4))
                xnp = S.enter_context(tc.tile_pool(name=f"xnp{ph}", bufs=3))
                dv = dst_mega[:].rearrange("p (k t) -> p k t", k=KC)
                for j in range(TJ):
                    xj = src_tiles_or_loader(j)
                    st = stp.tile([P, 2, 6], F32, tag="st")
                    xr2 = xj[:].rearrange("p (g f) -> p g f", f=NF)
                    for g in range(2):
                        nc.vector.bn_stats(out=st[:, g, :], in_=xr2[:, g, :])
                    mv = stp.tile([P, 2], F32, tag="mv")
                    nc.vector.bn_aggr(out=mv[:], in_=st[:])
                    srt = stp.tile([P, 1], F32, tag="srt")
                    nc.scalar.activation(out=srt[:], in_=mv[:, 1:2],
                                         func=ACTF.Sqrt, bias=epsc[:], scale=1.0)
                    rc = stp.tile([P, 1], F32, tag="rc")
                    nc.vector.reciprocal(rc[:], srt[:])
                    xnr = xnp.tile([P, C], BF16, tag="xnr")
                    nc.vector.tensor_scalar(
                        out=xnr[:], in0=xj[:], scalar1=mv[:, 0:1], scalar2=rc[:],
                        op0=ALU.subtract, op1=ALU.mult)
                    for kb in range(2):
                        if kb == 0:
                            pt = sptile([P, 4 * P], BF16, "pt")
                        else:
                            pt = psum.tile([P, 4 * P], BF16,
                                           tag=f"PA{_pa[0] % 4}", name="ptb")
                            _pa[0] += 1
                        for k4 in range(4):
                            k = kb * 4 + k4
                            nc.tensor.transpose(pt[:, k4 * P:(k4 + 1) * P],
                                                xnr[:, k * P:(k + 1) * P],
                                                identb[:])
                        dst = dv[:, kb * 4:(kb + 1) * 4, j * P:(j + 1) * P]
                        src = pt[:].rearrange("p (a b) -> p a b", a=4)
                        if kb == 0:
                            nc.vector.tensor_copy(out=dst, in_=src)
                        else:
                            nc.scalar.activation(out=dst, in_=src,
                                                 func=ACTF.Copy)

        xrp = ES.enter_context(tc.tile_pool(name="xrp", bufs=3))

        def load_x(j):
            xj = xrp.tile([P, C], F32, tag="xrow")
            row_dma(out=xj[:], in_=x_d[j * P:(j + 1) * P, :])
            return xj

        with (tc.For_i(0, hwloop, 1) if hwloop else nullcontext()):
            for _it in range(loop):
                # ---------- weights: issue all DMAs up front (prefetch) ----------
                wq_sb = wtiles("wq")
                wk_sb = wtiles("wk")
                wv_sb = wtiles("wv")
                # w1/w2 reuse wv/wq slots (dead after QKV); DMAs self-order
                w1_sb = wtiles("w1", share="wv")
                w2_sb = [arena.tile([P, C], BF16,
                                    tag=(f"wq_{k}" if k < QC else f"wk_{k - QC}"),
                                    name=f"w2{k}") for k in range(KC)]
                for q in range(QC):
                    for nm, tl in (("wq", wq_sb), ("wk", wk_sb), ("wv", wv_sb)):
                        bulk_dma(out=tl[q][:], in_=w_d[nm][q * P:(q + 1) * P, :])
                # proj weights: plain bf16 [P, C] per c'-chunk
                wp_sb = [arena.tile([P, C], BF16, tag=f"wp_{k}", name=f"wp{k}")
                         for k in range(KC)]
                for k in range(KC):
                    bulk_dma(out=wp_sb[k][:], in_=wp_d[k * P:(k + 1) * P, :])

                # ---------- Phase 0: LN1 (stats + normalize + transpose) ----------
                xn8 = mega("XN1")
                ln_pass(load_x, "xn", xn8, 0)
                xnv = xn8[:].rearrange("p (k t) -> p k t", k=KC)

                if phases >= 2:
                    # ---------------- Phase 1: QKV (fp8 DoubleRow) ----------------
                    QT = [arena.tile([P, T], F8, tag=f"QT_{m}", name=f"QT{m}")
                          for m in range(KC)]
                    KTt = [arena.tile([P, T], F8, tag=f"KT_{m}", name=f"KT{m}")
                           for m in range(KC)]
                    # tn pairs back-to-back per weight chunk: each LDWEIGHTS
                    # serves two matmuls (LDW elision / pull-ahead overlap)
                    for m in range(KC):
                        for w_sb, QK, col in ((wq_sb, QT, "q"), (wk_sb, KTt, "k")):
                            pq0 = patile(nm="pq0")
                            pq1 = patile(nm="pq1")
                            for q in range(QC):
                                for tn, pq in ((0, pq0), (1, pq1)):
                                    nc.tensor.matmul(
                                        pq[:],
                                        lhsT=w_sb[q][:, :, m * P:(m + 1) * P],
                                        rhs=xnv[:, 2 * q:2 * q + 2,
                                                slice(tn * NF, (tn + 1) * NF)],
                                        start=(q == 0), stop=(q == QC - 1),
                                        perf_mode=DR)
                            for tn, pq in ((0, pq0), (1, pq1)):
                                nc.vector.tensor_scalar(
                                    out=QK[m][:, tn * NF:(tn + 1) * NF],
                                    in0=pq[:], scalar1=QS / WS,
                                    scalar2=cols[col][:, m:m + 1],
                                    op0=ALU.mult, op1=ALU.add)

                    V = [arena.tile([P, H, Dh + 1], BF16, tag=f"V_{j}",
                                    name=f"V{j}") for j in range(TJ)]
                    for j in range(TJ):
                        nc.gpsimd.memset(V[j][:, :, Dh:Dh + 1], VIS)
                        pv0 = patile(nm="pv0")
                        pv1 = patile(nm="pv1")
                        nc.tensor.matmul(pv0[:], lhsT=onesb[0:1, :],
                                         rhs=bvr[0:1, 0:NF],
                                         start=True, stop=False)
                        nc.tensor.matmul(pv1[:], lhsT=onesb[0:1, :],
                                         rhs=bvr[0:1, NF:C],
                                         start=True, stop=False)
                        for q in range(QC):
                            xsl = xnv[:, 2 * q:2 * q + 2, j * P:(j + 1) * P]
                            for hn, pv in ((0, pv0), (1, pv1)):
                                nc.tensor.matmul(
                                    pv[:], lhsT=xsl,
                                    rhs=wv_sb[q][:, :,
                                                 hn * NF:(hn + 1) * NF],
                                    start=False, stop=(q == QC - 1),
                                    perf_mode=DR)
                        for hn, pv in ((0, pv0), (1, pv1)):
                            nc.scalar.activation(
                                out=V[j][:, hn * 8:(hn + 1) * 8, 0:Dh],
                                in_=pv[:].rearrange("p (h d) -> p h d", d=Dh),
                                func=ACTF.Identity, scale=1.0 / WS)
                    # late weights into the now-free wv/wq/wk slots
                    for q in range(QC):
                        bulk_dma(out=w1_sb[q][:],
                                 in_=w_d["w1"][q * P:(q + 1) * P, :])
                    for k in range(KC):
                        bulk_dma(out=w2_sb[k][:],
                                 in_=w2_d[k * P:(k + 1) * P, :])

                # proj residual rows (x + bproj) prefetched before attention
                # so proj can start the moment its attnT half is ready
                xrb = [arena.tile([P, C], BF16, tag=f"XRB_{j}", name=f"xrb{j}")
                       for j in range(TJ)]
                for j in range(TJ):
                    xj2 = load_x(j)
                    nc.gpsimd.tensor_tensor(xrb[j][:], xj2[:], bpb[:], ALU.add)

                attnT = arena.tile([P, KC * T], BF16, tag="ATT", name="attnT")
                atv = attnT[:].rearrange("p (m t) -> p m t", m=KC)
                if phases >= 3:
                    # ---------------- Phase 2: attention ----------------
                    with ExitStack() as S:
                        ep = S.enter_context(tc.tile_pool(name="ep", bufs=3))
                        rp = S.enter_context(tc.tile_pool(name="rp", bufs=3))
                        bp_ = S.enter_context(tc.tile_pool(name="bp", bufs=2))
                        tp1 = S.enter_context(tc.tile_pool(name="tp1", bufs=2))
                        for tn in range(TN):
                            tsl = slice(tn * NF, (tn + 1) * NF)
                            i_hi = 4 * (tn + 1)
                            for m in range(KC):
                                h0, h1 = 2 * m, 2 * m + 1
                                pa0 = patile((Dh + 1, NF), "pa0")
                                pa1 = patile((Dh + 1, NF), "pa1")
                                for i in range(i_hi):
                                    diag = i - 4 * tn
                                    d0 = max(diag, 0) * P
                                    esl = slice(d0, NF)
                                    qsl = slice(tn * NF + d0, (tn + 1) * NF)
                                    ssl = slice(i * P, (i + 1) * P)
                                    sp2 = sptile([P, 2, NF], F32, "s")
                                    nc.tensor.matmul(
                                        sp2[:, 0, esl], lhsT=KTt[m][0:64, ssl],
                                        rhs=QT[m][0:64, qsl],
                                        start=True, stop=True)
                                    nc.tensor.matmul(
                                        sp2[:, 1, esl], lhsT=KTt[m][64:128, ssl],
                                        rhs=QT[m][64:128, qsl],
                                        start=True, stop=True)
                                    Et = ep.tile([P, 2, NF], BF16, tag="E")
                                    nc.scalar.activation(
                                        out=Et[:, :, esl], in_=sp2[:, :, esl],
                                        func=ACTF.Exp, scale=SEXP)
                                    if diag >= 0:
                                        dsl = slice(d0, d0 + P)
                                        tri_b = bass.AP(
                                            tensor=tri01[:].tensor,
                                            offset=tri01[:].offset,
                                            ap=[tri01[:].ap[0], [0, 2],
                                                *tri01[:].ap[1:]])
                                        nc.vector.tensor_tensor(
                                            Et[:, :, dsl], Et[:, :, dsl],
                                            tri_b, ALU.mult)
                                    nc.tensor.matmul(
                                        pa0[:, esl], lhsT=V[i][:, h0, :],
                                        rhs=Et[:, 0, esl],
                                        start=(i == 0), stop=(i == i_hi - 1))
                                    nc.tensor.matmul(
                                        pa1[:, esl], lhsT=V[i][:, h1, :],
                                        rhs=Et[:, 1, esl],
                                        start=(i == 0), stop=(i == i_hi - 1))
                                # evacuate pa to SBUF immediately (frees the
                                # PSUM banks from the denominator-bounce
                                # latency), then: reshape sums to [128, 8] via
                                # DMA so the bit-exact reciprocal runs wide,
                                # bounce through DRAM for the broadcast, and
                                # normalize on the idle Pool engine.
                                av0 = rp.tile([Dh + 1, NF], F32, tag="av0")
                                av1 = rp.tile([Dh + 1, NF], F32, tag="av1")
                                nc.vector.tensor_copy(out=av0[:], in_=pa0[:])
                                nc.vector.tensor_copy(out=av1[:], in_=pa1[:])
                                s2 = rp.tile([P, 8], F32, tag="s2")
                                nc.gpsimd.dma_start(
                                    out=s2[:, 0:4],
                                    in_=av0[Dh:Dh + 1, :])
                                nc.gpsimd.dma_start(
                                    out=s2[:, 4:8],
                                    in_=av1[Dh:Dh + 1, :])
                                nc.vector.reciprocal(s2[:], s2[:])
                                drr = dram.tile([2 * NF], F32, tag="rsums")
                                nc.gpsimd.dma_start(
                                    out=drr[0:NF].rearrange("(p i) -> p i", i=4),
                                    in_=s2[:, 0:4])
                                nc.gpsimd.dma_start(
                                    out=drr[NF:2 * NF].rearrange(
                                        "(p i) -> p i", i=4),
                                    in_=s2[:, 4:8])
                                bct = bp_.tile([Dh, 2, NF], F32, tag="bct")
                                nc.sync.dma_start(
                                    out=bct[:, 0, :],
                                    in_=drr[0:NF][None, :].to_broadcast([Dh, NF]))
                                nc.sync.dma_start(
                                    out=bct[:, 1, :],
                                    in_=drr[NF:2 * NF][None, :].to_broadcast(
                                        [Dh, NF]))
                                nc.gpsimd.tensor_tensor(
                                    atv[0:Dh, m, tsl], av0[0:Dh, :],
                                    bct[:, 0, :], ALU.mult)
                                tmp1 = tp1.tile([Dh, NF], BF16, tag="t1")
                                nc.gpsimd.tensor_tensor(
                                    tmp1[:], av1[0:Dh, :], bct[:, 1, :], ALU.mult)
                                nc.gpsimd.dma_start(
                                    out=atv[Dh:2 * Dh, m, tsl], in_=tmp1[:])

                y_n = [arena.tile([P, C], BF16, tag=f"Y_{j}", name=f"y{j}")
                       for j in range(TJ)]
                if phases >= 4:
                    # ---------- Phase 3: proj + residual -> y (bf16, normal) -----
                    for j in range(TJ):
                        for nn in range(TN):
                            csl = slice(nn * NF, (nn + 1) * NF)
                            pp = patile(nm="pp")
                            for k in range(KC):
                                nc.tensor.matmul(
                                    pp[:], lhsT=atv[:, k, j * P:(j + 1) * P],
                                    rhs=wp_sb[k][:, csl],
                                    start=(k == 0), stop=(k == KC - 1))
                            nc.vector.tensor_tensor(
                                y_n[j][:, csl], pp[:], xrb[j][:, csl], ALU.add)

                if phases >= 5:
                    # ---------------- Phase 4: LN2 ----------------
                    xn28 = mega("XN2")
                    ln_pass(lambda j: y_n[j], "xn2", xn28, 1)
                    xn2v = xn28[:].rearrange("p (k t) -> p k t", k=KC)

                hT = arena.tile([P, KC * T], BF16, tag="HT", name="hT")
                htv = hT[:].rearrange("p (m t) -> p m t", m=KC)
                if phases >= 6:
                    # ---------------- Phase 5: MLP fc1 + relu ----------------
                    for m in range(KC):
                        ph0 = patile(nm="ph0")
                        ph1 = patile(nm="ph1")
                        for q in range(QC):
                            for tn, ph in ((0, ph0), (1, ph1)):
                                nc.tensor.matmul(
                                    ph[:], lhsT=w1_sb[q][:, :, m * P:(m + 1) * P],
                                    rhs=xn2v[:, 2 * q:2 * q + 2,
                                             slice(tn * NF, (tn + 1) * NF)],
                                    start=(q == 0), stop=(q == QC - 1),
                                    perf_mode=DR)
                        for tn, ph in ((0, ph0), (1, ph1)):
                            nc.scalar.activation(
                                out=htv[:, m, tn * NF:(tn + 1) * NF], in_=ph[:],
                                func=ACTF.Relu,
                                bias=cols["b1"][:, m:m + 1], scale=HS / WS)

                if phases >= 7:
                    # ---------- Phase 6: MLP fc2 + residual -> out ----------
                    with ExitStack() as S:
                        otp = S.enter_context(tc.tile_pool(name="otp", bufs=3))
                        y2p = S.enter_context(tc.tile_pool(name="y2p", bufs=2))
                        for j in range(TJ):
                            y2 = y2p.tile([P, C], BF16, tag="y2")
                            nc.gpsimd.tensor_tensor(y2[:], y_n[j][:], b2b[:],
                                                    ALU.add)
                            for nn in range(TN):
                                csl = slice(nn * NF, (nn + 1) * NF)
                                po = patile(nm="po")
                                for k in range(KC):
                                    nc.tensor.matmul(
                                        po[:], lhsT=htv[:, k, j * P:(j + 1) * P],
                                        rhs=w2_sb[k][:, csl],
                                        start=(k == 0), stop=(k == KC - 1))
                                ot = otp.tile([P, NF], F32, tag="ot")
                                nc.vector.scalar_tensor_tensor(
                                    out=ot[:], in0=po[:], scalar=1.0 / HS,
                                    in1=y2[:, csl], op0=ALU.mult, op1=ALU.add)
                                bulk_dma(out=out_d[j * P:(j + 1) * P, csl],
                                         in_=ot[:])

    nc.compile()
    return nc


def _f8(a):
    return np.clip(a, -240.0, 240.0).astype(F8NP)


def _pack_dr(w_eff):
    """[C, M] effective weight -> [QC*P, 2*M] fp8 DoubleRow layout
    (row q*128+p, col i*M+m  <-  w_eff[q*256 + i*128 + p, m] * WS)."""
    M = w_eff.shape[1]
    w = (w_eff * WS).reshape(QC, 2, P, M).transpose(0, 2, 1, 3).reshape(
        QC * P, 2 * M)
    return _f8(np.ascontiguousarray(w))


def _prep_inputs(inputs):
    """Host-side weight repacking/quantization; returns per-core in_maps."""
    f = np.float32
    x = np.ascontiguousarray(np.asarray(inputs["x"], dtype=f))        # [B, T, C]
    g1 = np.asarray(inputs["g1"], dtype=f)
    be1 = np.asarray(inputs["beta1"], dtype=f)
    g2 = np.asarray(inputs["g2"], dtype=f)
    be2 = np.asarray(inputs["beta2"], dtype=f)

    wq = np.asarray(inputs["Wq"], dtype=f).transpose(1, 0, 2).reshape(C, C)
    wk = np.asarray(inputs["Wk"], dtype=f).transpose(1, 0, 2).reshape(C, C)
    wv = np.asarray(inputs["Wv"], dtype=f).transpose(1, 0, 2).reshape(C, C)
    wp = np.asarray(inputs["Wproj"], dtype=f)
    w1 = np.asarray(inputs["W1"], dtype=f)
    w2 = np.asarray(inputs["W2"], dtype=f)
    b1 = np.asarray(inputs["b1"], dtype=f)

    common = {
        "wq": _pack_dr(g1[:, None] * wq),
        "wk": _pack_dr(g1[:, None] * wk),
        "wv": _pack_dr(g1[:, None] * wv),
        "wp": np.ascontiguousarray(wp).astype(BF16NP),
        "w1": _pack_dr(g2[:, None] * w1),
        "w2": np.ascontiguousarray(w2).astype(BF16NP),
        "colq": np.ascontiguousarray(
            (QS * (be1 @ wq)).reshape(KC, P).T),
        "colk": np.ascontiguousarray(
            (QS * (be1 @ wk)).reshape(KC, P).T),
        "b1c": np.ascontiguousarray(
            (HS * (b1 + be2 @ w1)).reshape(KC, P).T),
        "bvr": (WS * (be1 @ wv)).astype(BF16NP),
        "bp": np.asarray(inputs["bproj"], dtype=f),
        "b2": np.asarray(inputs["b2"], dtype=f),
    }
    return [{"x": x[b], **common} for b in range(N_CORES)]


def kernel(**inputs) -> np.ndarray:
    if "nc" not in _CACHE:
        _CACHE["nc"] = build_nc()
    nc = _CACHE["nc"]
    in_maps = _prep_inputs(inputs)
    res = run_bass_kernel_spmd(nc, in_maps, list(range(N_CORES)))
    out = np.stack([res.results[b]["out"] for b in range(N_CORES)], axis=0)
    return out.astype(np.float32)


if __name__ == "__main__":
    rng = np.random.default_rng(0)
    demo = {
        "x": rng.standard_normal((B, T, C), dtype=np.float32),
        "Wq": rng.standard_normal((H, C, Dh), dtype=np.float32) * 0.02,
        "Wk": rng.standard_normal((H, C, Dh), dtype=np.float32) * 0.02,
        "Wv": rng.standard_normal((H, C, Dh), dtype=np.float32) * 0.02,
        "Wproj": rng.standard_normal((C, C), dtype=np.float32) * 0.02,
        "bproj": np.zeros(C, np.float32),
        "W1": rng.standard_normal((C, C), dtype=np.float32) * 0.02,
        "b1": np.zeros(C, np.float32),
        "W2": rng.standard_normal((C, C), dtype=np.float32) * 0.02,
        "b2": np.zeros(C, np.float32),
        "g1": np.ones(C, np.float32),
        "beta1": np.zeros(C, np.float32),
        "g2": np.ones(C, np.float32),
        "beta2": np.zeros(C, np.float32),
    }
    y = kernel(**demo)
    print("out", y.shape, y.dtype, float(np.abs(y).max()))


# revision 44
# speedup vs baseline: 1.8734x; 1.8734x over previous
"""Trainium2 Bass kernel for nn_Block_3616362463321 (dense transformer block).

B=8, T=1024, C=1024, H=16, Dh=64. Data-parallel over batch: core b gets x[b].
Weights replicated to all 8 cores; no collectives.

v2 design (vs baseline):
  - All six weight GEMMs (QKV / proj / fc1 / fc2) run fp8e4m3 with
    perf_mode=DoubleRow: K=256 contraction per pass, 0.5 cycles/row.
    Weights are host-quantized (x1024, clip +-240) into an interleaved
    [q, p, islot, m] layout; activations are quantized on the fly into
    "mega" SBUF tiles [128, 8*1024] fp8 whose (k=c-chunk, t) layout serves
    both the DoubleRow moving-operand view [p, 2, t] and the stationary
    view [p, 2, 128].
  - LayerNorm statistics AND normalization happen in natural [t, c] layout
    (per-partition mean/rstd -> one tensor_scalar), with gamma folded into
    the weights host-side and beta folded into per-output bias columns.
    No DRAM stat bounces.  Normalized activations are PE-transposed as fp8
    (1 cycle/row), 4 blocks batched per PSUM bank.
  - Attention keeps the baseline S^T orientation (fp8 Q/K, bf16 E,
    fp8 V with fused 0.125-ones column for softmax denominators), with:
      * S row-tile pairs (K=64 at rows 0/64) writing one 2-bank PSUM pair
        tile -> a single paired exp per (m, tn, i) on ACT,
      * exact causal trims everywhere (bf16/fp8 matmuls have no N>=256
        restriction),
      * causal masking as a post-exp bf16 triangle multiply (2x DVE mode),
      * reciprocal_approx_fast for denominators + SBUF->SBUF broadcast DMA,
  - Residual/dequant fused into single scalar_tensor_tensor ops; SBUF-only
    elementwise work (x+bias rows) offloaded to the idle GpSimd engine.
  - Bulk DMA spread over the SP / Pool / ACT hardware queues.
"""
import sys

sys.path.insert(0, "/opt/trn_rl_repo")

from contextlib import ExitStack, nullcontext

import numpy as np
import ml_dtypes

import concourse.bacc as bacc
import concourse.bass as bass
import concourse.mybir as mybir
import concourse.tile as tile
from concourse.bass_utils import run_bass_kernel_spmd

P = 128
B, T, C, H = 8, 1024, 1024, 16
Dh = C // H            # 64
EPS = 1e-5
NF = 512               # matmul moving free dim (fp32 PSUM bank limit)
KC = C // P            # 8 c-chunks of 128
QC = C // 256          # 4 c-chunks of 256 (DoubleRow)
TJ = T // P            # 8 t-chunks of 128
TN = T // NF           # 2 t-chunks of 512
F32 = mybir.dt.float32
F32R = mybir.dt.float32r
BF16 = mybir.dt.bfloat16
F8 = mybir.dt.float8e4
ALU = mybir.AluOpType
ACTF = mybir.ActivationFunctionType
DR = mybir.MatmulPerfMode.DoubleRow

WS = 1024.0            # host weight upscale (fp8 range use)
QS = 8.0               # Q/K storage scale
SEXP = (Dh ** -0.5) / (QS * QS)   # exp scale absorbing Q/K storage scales
VIS = 1.0              # V ones-column value (bf16 attnT: true scale)
HS = 8.0               # hT storage scale

N_CORES = 8

_CACHE = {}

F8NP = ml_dtypes.float8_e4m3
BF16NP = ml_dtypes.bfloat16


def _bcast_row_ap(handle_ap, parts):
    """AP reading a [N]-shaped DRAM tensor broadcast across `parts` partitions."""
    return bass.AP(
        tensor=handle_ap.tensor,
        offset=handle_ap.offset,
        ap=[[0, parts], *handle_ap.ap],
    )


def build_nc(loop=1, hwloop=0, phases=7):
    nc = bacc.Bacc("TRN2", target_bir_lowering=False, debug=False)

    x_d = nc.dram_tensor("x", [T, C], F32, kind="ExternalInput")
    w_d = {}
    for nm in ("wq", "wk", "wv", "w1"):
        w_d[nm] = nc.dram_tensor(nm, [QC * P, 2 * C], F8, kind="ExternalInput")
    wp_d = nc.dram_tensor("wp", [C, C], BF16, kind="ExternalInput")
    w2_d = nc.dram_tensor("w2", [C, C], BF16, kind="ExternalInput")
    colq_d = nc.dram_tensor("colq", [C], F32, kind="ExternalInput")
    colk_d = nc.dram_tensor("colk", [C], F32, kind="ExternalInput")
    b1c_d = nc.dram_tensor("b1c", [C], F32, kind="ExternalInput")
    bvr_d = nc.dram_tensor("bvr", [C], BF16, kind="ExternalInput")
    bp_d = nc.dram_tensor("bp", [C], F32, kind="ExternalInput")
    b2_d = nc.dram_tensor("b2", [C], F32, kind="ExternalInput")
    out_d = nc.dram_tensor("out", [T, C], F32, kind="ExternalOutput")

    identb_c = nc.inline_tensor(np.eye(P).astype(BF16NP), name="identb_c")
    # post-exp causal keep-mask for a diagonal [s,t] block: keep where s <= t
    tri_np = (np.arange(P)[:, None] <= np.arange(P)[None, :]).astype(BF16NP)
    tri_c = nc.inline_tensor(tri_np, name="tri_c")

    with tile.TileContext(nc) as tc, ExitStack() as ES:
        singles = ES.enter_context(tc.tile_pool(name="singles", bufs=1))
        dram = ES.enter_context(tc.tile_pool(name="drsc", bufs=1, space="DRAM"))

        identb = singles.tile([P, P], BF16)
        nc.sync.dma_start(out=identb[:], in_=identb_c.ap())
        tri01 = singles.tile([P, P], BF16)
        nc.sync.dma_start(out=tri01[:], in_=tri_c.ap())
        epsc = singles.tile([P, 1], F32)
        nc.vector.memset(epsc[:], EPS)
        onesb = singles.tile([1, P], BF16)
        nc.vector.memset(onesb[:], 1.0)

        # bias columns [P, KC]: element (p, k) = vec[k*P + p]
        cols = {}

        def load_col(nm, hd):
            t_ = singles.tile([P, KC], F32, tag=f"col_{nm}", name=f"col_{nm}")
            nc.sync.dma_start(out=t_[:], in_=hd.ap().rearrange("(k p) -> p k", p=P))
            cols[nm] = t_

        load_col("q", colq_d)
        load_col("k", colk_d)
        load_col("b1", b1c_d)
        bvr = singles.tile([1, C], BF16)
        nc.gpsimd.dma_start(out=bvr[:], in_=bvr_d.ap()[None, :])
        bpb = singles.tile([P, C], F32)
        nc.gpsimd.dma_start(out=bpb[:], in_=_bcast_row_ap(bp_d.ap(), P))
        b2b = singles.tile([P, C], F32)
        nc.gpsimd.dma_start(out=b2b[:], in_=_bcast_row_ap(b2_d.ap(), P))

        # ---- SBUF arena ----
        arena = ES.enter_context(tc.tile_pool(name="arena", bufs=1))

        def mega(tag):
            return arena.tile([P, KC * T], F8, tag=tag, name=tag)

        # weight tiles: [P, 2, C] fp8 per 256-chunk
        def wtiles(nm, share=None):
            tg = share or nm
            return [arena.tile([P, 2, C], F8, tag=f"{tg}_{q}", name=f"{nm}{q}")
                    for q in range(QC)]

        # ---- PSUM pool: declare SP pair tags first (2 banks each), then PA ----
        psum = ES.enter_context(tc.tile_pool(name="psum", bufs=1, space="PSUM"))
        _sp = [0]
        _pa = [0]

        def sptile(shape, dtype, nm="sp"):
            t = psum.tile(list(shape), dtype, tag=f"SP{_sp[0] % 2}",
                          name=f"{nm}{_sp[0]}")
            _sp[0] += 1
            return t

        def patile(shape=(P, NF), nm="pa"):
            t = psum.tile(list(shape), F32, tag=f"PA{_pa[0] % 4}",
                          name=f"{nm}{_pa[0]}")
            _pa[0] += 1
            return t

        # force tag declaration order: SP0, SP1 as [P, 2, NF] f32 (2 banks each)
        _ = psum.tile([P, 2, NF], F32, tag="SP0", name="spdecl0")
        _ = psum.tile([P, 2, NF], F32, tag="SP1", name="spdecl1")

        # weights on the Pool/ACT queues; x and out rows keep SP (+ACT) free-ish
        _dq = [0]
        _dqe = [nc.gpsimd, nc.scalar]

        def bulk_dma(out, in_):
            eng = _dqe[_dq[0] % len(_dqe)]
            _dq[0] += 1
            eng.dma_start(out=out, in_=in_)

        def row_dma(out, in_):
            nc.sync.dma_start(out=out, in_=in_)

        def ln_pass(src_tiles_or_loader, xn_tag, dst_mega, ph, out_rows_dtype=F8):
            """Natural-layout LN: per 128-row chunk j, bn_stats -> mean/rstd
            columns -> one tensor_scalar into an fp8 row tile -> PE-transpose
            (fp8, 4 blocks per PSUM batch) into dst_mega [(k t)] layout."""
            with ExitStack() as S:
                stp = S.enter_context(tc.tile_pool(name=f"stp{ph}", bufs=4))
                xnp = S.enter_context(tc.tile_pool(name=f"xnp{ph}", bufs=3))
                dv = dst_mega[:].rearrange("p (k t) -> p k t", k=KC)
                for j in range(TJ):
                    xj = src_tiles_or_loader(j)
                    st = stp.tile([P, 2, 6], F32, tag="st")
                    xr2 = xj[:].rearrange("p (g f) -> p g f", f=NF)
                    for g in range(2):
                        nc.vector.bn_stats(out=st[:, g, :], in_=xr2[:, g, :])
                    mv = stp.tile([P, 2], F32, tag="mv")
                    nc.vector.bn_aggr(out=mv[:], in_=st[:])
                    srt = stp.tile([P, 1], F32, tag="srt")
                    nc.scalar.activation(out=srt[:], in_=mv[:, 1:2],
                                         func=ACTF.Sqrt, bias=epsc[:], scale=1.0)
                    rc = stp.tile([P, 1], F32, tag="rc")
                    nc.vector.reciprocal(rc[:], srt[:])
                    xnr = xnp.tile([P, C], BF16, tag="xnr")
                    nc.vector.tensor_scalar(
                        out=xnr[:], in0=xj[:], scalar1=mv[:, 0:1], scalar2=rc[:],
                        op0=ALU.subtract, op1=ALU.mult)
                    for kb in range(2):
                        pt = sptile([P, 4 * P], BF16, "pt")
                        for k4 in range(4):
                            k = kb * 4 + k4
                            nc.tensor.transpose(pt[:, k4 * P:(k4 + 1) * P],
                                                xnr[:, k * P:(k + 1) * P],
                                                identb[:])
                        dst = dv[:, kb * 4:(kb + 1) * 4, j * P:(j + 1) * P]
                        src = pt[:].rearrange("p (a b) -> p a b", a=4)
                        if kb == 0:
                            nc.vector.tensor_copy(out=dst, in_=src)
                        else:
                            nc.scalar.activation(out=dst, in_=src,
                                                 func=ACTF.Copy)

        xrp = ES.enter_context(tc.tile_pool(name="xrp", bufs=3))

        def load_x(j):
            xj = xrp.tile([P, C], F32, tag="xrow")
            row_dma(out=xj[:], in_=x_d[j * P:(j + 1) * P, :])
            return xj

        with (tc.For_i(0, hwloop, 1) if hwloop else nullcontext()):
            for _it in range(loop):
                # ---------- weights: issue all DMAs up front (prefetch) ----------
                wq_sb = wtiles("wq")
                wk_sb = wtiles("wk")
                wv_sb = wtiles("wv")
                # w1/w2 reuse wv/wq slots (dead after QKV); DMAs self-order
                w1_sb = wtiles("w1", share="wv")
                w2_sb = [arena.tile([P, C], BF16,
                                    tag=(f"wq_{k}" if k < QC else f"wk_{k - QC}"),
                                    name=f"w2{k}") for k in range(KC)]
                for q in range(QC):
                    for nm, tl in (("wq", wq_sb), ("wk", wk_sb), ("wv", wv_sb)):
                        bulk_dma(out=tl[q][:], in_=w_d[nm][q * P:(q + 1) * P, :])
                # proj weights: plain bf16 [P, C] per c'-chunk
                wp_sb = [arena.tile([P, C], BF16, tag=f"wp_{k}", name=f"wp{k}")
                         for k in range(KC)]
                for k in range(KC):
                    bulk_dma(out=wp_sb[k][:], in_=wp_d[k * P:(k + 1) * P, :])

                # ---------- Phase 0: LN1 (stats + normalize + transpose) ----------
                xn8 = mega("XN1")
                ln_pass(load_x, "xn", xn8, 0)
                xnv = xn8[:].rearrange("p (k t) -> p k t", k=KC)

                if phases >= 2:
                    # ---------------- Phase 1: QKV (fp8 DoubleRow) ----------------
                    QT = [arena.tile([P, T], F8, tag=f"QT_{m}", name=f"QT{m}")
                          for m in range(KC)]
                    KTt = [arena.tile([P, T], F8, tag=f"KT_{m}", name=f"KT{m}")
                           for m in range(KC)]
                    for tn in range(TN):
                        tsl = slice(tn * NF, (tn + 1) * NF)
                        for m in range(KC):
                            pq = patile(nm="pq")
                            for q in range(QC):
                                nc.tensor.matmul(
                                    pq[:], lhsT=wq_sb[q][:, :, m * P:(m + 1) * P],
                                    rhs=xnv[:, 2 * q:2 * q + 2, tsl],
                                    start=(q == 0), stop=(q == QC - 1),
                                    perf_mode=DR)
                            nc.vector.tensor_scalar(
                                out=QT[m][:, tsl], in0=pq[:], scalar1=QS / WS,
                                scalar2=cols["q"][:, m:m + 1],
                                op0=ALU.mult, op1=ALU.add)
                            pk = patile(nm="pk")
                            for q in range(QC):
                                nc.tensor.matmul(
                                    pk[:], lhsT=wk_sb[q][:, :, m * P:(m + 1) * P],
                                    rhs=xnv[:, 2 * q:2 * q + 2, tsl],
                                    start=(q == 0), stop=(q == QC - 1),
                                    perf_mode=DR)
                            nc.scalar.activation(
                                out=KTt[m][:, tsl], in_=pk[:], func=ACTF.Identity,
                                scale=QS / WS, bias=cols["k"][:, m:m + 1])

                    V = [arena.tile([P, H, Dh + 1], BF16, tag=f"V_{j}",
                                    name=f"V{j}") for j in range(TJ)]
                    for j in range(TJ):
                        nc.gpsimd.memset(V[j][:, :, Dh:Dh + 1], VIS)
                        for hn in range(TN):
                            hsl = slice(hn * NF, (hn + 1) * NF)
                            pv = patile(nm="pv")
                            nc.tensor.matmul(pv[:], lhsT=onesb[0:1, :],
                                             rhs=bvr[0:1, hsl],
                                             start=True, stop=False)
                            for q in range(QC):
                                nc.tensor.matmul(
                                    pv[:], lhsT=xnv[:, 2 * q:2 * q + 2,
                                                    j * P:(j + 1) * P],
                                    rhs=wv_sb[q][:, :, hsl],
                                    start=False, stop=(q == QC - 1),
                                    perf_mode=DR)
                            nc.scalar.activation(
                                out=V[j][:, hn * 8:(hn + 1) * 8, 0:Dh],
                                in_=pv[:].rearrange("p (h d) -> p h d", d=Dh),
                                func=ACTF.Identity, scale=1.0 / WS)
                    # late weights into the now-free wv/wq/wk slots
                    for q in range(QC):
                        bulk_dma(out=w1_sb[q][:],
                                 in_=w_d["w1"][q * P:(q + 1) * P, :])
                    for k in range(KC):
                        bulk_dma(out=w2_sb[k][:],
                                 in_=w2_d[k * P:(k + 1) * P, :])

                # proj residual rows (x + bproj) prefetched before attention
                # so proj can start the moment its attnT half is ready
                xrb = [arena.tile([P, C], BF16, tag=f"XRB_{j}", name=f"xrb{j}")
                       for j in range(TJ)]
                for j in range(TJ):
                    xj2 = load_x(j)
                    nc.gpsimd.tensor_tensor(xrb[j][:], xj2[:], bpb[:], ALU.add)

                attnT = arena.tile([P, KC * T], BF16, tag="ATT", name="attnT")
                atv = attnT[:].rearrange("p (m t) -> p m t", m=KC)
                if phases >= 3:
                    # ---------------- Phase 2: attention ----------------
                    with ExitStack() as S:
                        ep = S.enter_context(tc.tile_pool(name="ep", bufs=3))
                        rp = S.enter_context(tc.tile_pool(name="rp", bufs=3))
                        bp_ = S.enter_context(tc.tile_pool(name="bp", bufs=2))
                        tp1 = S.enter_context(tc.tile_pool(name="tp1", bufs=2))
                        for tn in range(TN):
                            tsl = slice(tn * NF, (tn + 1) * NF)
                            i_hi = 4 * (tn + 1)
                            for m in range(KC):
                                h0, h1 = 2 * m, 2 * m + 1
                                pa0 = patile((Dh + 1, NF), "pa0")
                                pa1 = patile((Dh + 1, NF), "pa1")
                                for i in range(i_hi):
                                    diag = i - 4 * tn
                                    d0 = max(diag, 0) * P
                                    esl = slice(d0, NF)
                                    qsl = slice(tn * NF + d0, (tn + 1) * NF)
                                    ssl = slice(i * P, (i + 1) * P)
                                    sp2 = sptile([P, 2, NF], F32, "s")
                                    nc.tensor.matmul(
                                        sp2[:, 0, esl], lhsT=KTt[m][0:64, ssl],
                                        rhs=QT[m][0:64, qsl],
                                        start=True, stop=True)
                                    nc.tensor.matmul(
                                        sp2[:, 1, esl], lhsT=KTt[m][64:128, ssl],
                                        rhs=QT[m][64:128, qsl],
                                        start=True, stop=True)
                                    Et = ep.tile([P, 2, NF], BF16, tag="E")
                                    nc.scalar.activation(
                                        out=Et[:, :, esl], in_=sp2[:, :, esl],
                                        func=ACTF.Exp, scale=SEXP)
                                    if diag >= 0:
                                        dsl = slice(d0, d0 + P)
                                        tri_b = bass.AP(
                                            tensor=tri01[:].tensor,
                                            offset=tri01[:].offset,
                                            ap=[tri01[:].ap[0], [0, 2],
                                                *tri01[:].ap[1:]])
                                        nc.vector.tensor_tensor(
                                            Et[:, :, dsl], Et[:, :, dsl],
                                            tri_b, ALU.mult)
                                    nc.tensor.matmul(
                                        pa0[:, esl], lhsT=V[i][:, h0, :],
                                        rhs=Et[:, 0, esl],
                                        start=(i == 0), stop=(i == i_hi - 1))
                                    nc.tensor.matmul(
                                        pa1[:, esl], lhsT=V[i][:, h1, :],
                                        rhs=Et[:, 1, esl],
                                        start=(i == 0), stop=(i == i_hi - 1))
                                # evacuate pa to SBUF immediately (frees the
                                # PSUM banks from the denominator-bounce
                                # latency), then: reshape sums to [128, 8] via
                                # DMA so the bit-exact reciprocal runs wide,
                                # bounce through DRAM for the broadcast, and
                                # normalize on the idle Pool engine.
                                av0 = rp.tile([Dh + 1, NF], F32, tag="av0")
                                av1 = rp.tile([Dh + 1, NF], F32, tag="av1")
                                nc.vector.tensor_copy(out=av0[:], in_=pa0[:])
                                nc.vector.tensor_copy(out=av1[:], in_=pa1[:])
                                s2 = rp.tile([P, 8], F32, tag="s2")
                                nc.gpsimd.dma_start(
                                    out=s2[:, 0:4],
                                    in_=av0[Dh:Dh + 1, :])
                                nc.gpsimd.dma_start(
                                    out=s2[:, 4:8],
                                    in_=av1[Dh:Dh + 1, :])
                                nc.vector.reciprocal(s2[:], s2[:])
                                drr = dram.tile([2 * NF], F32, tag="rsums")
                                nc.gpsimd.dma_start(
                                    out=drr[0:NF].rearrange("(p i) -> p i", i=4),
                                    in_=s2[:, 0:4])
                                nc.gpsimd.dma_start(
                                    out=drr[NF:2 * NF].rearrange(
                                        "(p i) -> p i", i=4),
                                    in_=s2[:, 4:8])
                                bct = bp_.tile([Dh, 2, NF], F32, tag="bct")
                                nc.sync.dma_start(
                                    out=bct[:, 0, :],
                                    in_=drr[0:NF][None, :].to_broadcast([Dh, NF]))
                                nc.sync.dma_start(
                                    out=bct[:, 1, :],
                                    in_=drr[NF:2 * NF][None, :].to_broadcast(
                                        [Dh, NF]))
                                nc.gpsimd.tensor_tensor(
                                    atv[0:Dh, m, tsl], av0[0:Dh, :],
                                    bct[:, 0, :], ALU.mult)
                                tmp1 = tp1.tile([Dh, NF], BF16, tag="t1")
                                nc.gpsimd.tensor_tensor(
                                    tmp1[:], av1[0:Dh, :], bct[:, 1, :], ALU.mult)
                                nc.gpsimd.dma_start(
                                    out=atv[Dh:2 * Dh, m, tsl], in_=tmp1[:])

                y_n = [arena.tile([P, C], BF16, tag=f"Y_{j}", name=f"y{j}")
                       for j in range(TJ)]
                if phases >= 4:
                    # ---------- Phase 3: proj + residual -> y (bf16, normal) -----
                    for j in range(TJ):
                        for nn in range(TN):
                            csl = slice(nn * NF, (nn + 1) * NF)
                            pp = patile(nm="pp")
                            for k in range(KC):
                                nc.tensor.matmul(
                                    pp[:], lhsT=atv[:, k, j * P:(j + 1) * P],
                                    rhs=wp_sb[k][:, csl],
                                    start=(k == 0), stop=(k == KC - 1))
                            nc.vector.tensor_tensor(
                                y_n[j][:, csl], pp[:], xrb[j][:, csl], ALU.add)

                if phases >= 5:
                    # ---------------- Phase 4: LN2 ----------------
                    xn28 = mega("XN2")
                    ln_pass(lambda j: y_n[j], "xn2", xn28, 1)
                    xn2v = xn28[:].rearrange("p (k t) -> p k t", k=KC)

                hT = arena.tile([P, KC * T], BF16, tag="HT", name="hT")
                htv = hT[:].rearrange("p (m t) -> p m t", m=KC)
                if phases >= 6:
                    # ---------------- Phase 5: MLP fc1 + relu ----------------
                    for tn in range(TN):
                        tsl = slice(tn * NF, (tn + 1) * NF)
                        for m in range(KC):
                            ph = patile(nm="ph")
                            for q in range(QC):
                                nc.tensor.matmul(
                                    ph[:], lhsT=w1_sb[q][:, :, m * P:(m + 1) * P],
                                    rhs=xn2v[:, 2 * q:2 * q + 2, tsl],
                                    start=(q == 0), stop=(q == QC - 1),
                                    perf_mode=DR)
                            nc.scalar.activation(
                                out=htv[:, m, tsl], in_=ph[:], func=ACTF.Relu,
                                bias=cols["b1"][:, m:m + 1], scale=HS / WS)

                if phases >= 7:
                    # ---------- Phase 6: MLP fc2 + residual -> out ----------
                    with ExitStack() as S:
                        otp = S.enter_context(tc.tile_pool(name="otp", bufs=3))
                        y2p = S.enter_context(tc.tile_pool(name="y2p", bufs=2))
                        for j in range(TJ):
                            y2 = y2p.tile([P, C], BF16, tag="y2")
                            nc.gpsimd.tensor_tensor(y2[:], y_n[j][:], b2b[:],
                                                    ALU.add)
                            for nn in range(TN):
                                csl = slice(nn * NF, (nn + 1) * NF)
                                po = patile(nm="po")
                                for k in range(KC):
                                    nc.tensor.matmul(
                                        po[:], lhsT=htv[:, k, j * P:(j + 1) * P],
                                        rhs=w2_sb[k][:, csl],
                                        start=(k == 0), stop=(k == KC - 1))
                                ot = otp.tile([P, NF], F32, tag="ot")
                                nc.vector.scalar_tensor_tensor(
                                    out=ot[:], in0=po[:], scalar=1.0 / HS,
                                    in1=y2[:, csl], op0=ALU.mult, op1=ALU.add)
                                bulk_dma(out=out_d[j * P:(j + 1) * P, csl],
                                         in_=ot[:])

    nc.compile()
    return nc


def _f8(a):
    return np.clip(a, -240.0, 240.0).astype(F8NP)


def _pack_dr(w_eff):
    """[C, M] effective weight -> [QC*P, 2*M] fp8 DoubleRow layout
    (row q*128+p, col i*M+m  <-  w_eff[q*256 + i*128 + p, m] * WS)."""
    M = w_eff.shape[1]
    w = (w_eff * WS).reshape(QC, 2, P, M).transpose(0, 2, 1, 3).reshape(
        QC * P, 2 * M)
    return _f8(np.ascontiguousarray(w))


def _prep_inputs(inputs):
    """Host-side weight repacking/quantization; returns per-core in_maps."""
    f = np.float32
    x = np.ascontiguousarray(np.asarray(inputs["x"], dtype=f))        # [B, T, C]
    g1 = np.asarray(inputs["g1"], dtype=f)
    be1 = np.asarray(inputs["beta1"], dtype=f)
    g2 = np.asarray(inputs["g2"], dtype=f)
    be2 = np.asarray(inputs["beta2"], dtype=f)

    wq = np.asarray(inputs["Wq"], dtype=f).transpose(1, 0, 2).reshape(C, C)
    wk = np.asarray(inputs["Wk"], dtype=f).transpose(1, 0, 2).reshape(C, C)
    wv = np.asarray(inputs["Wv"], dtype=f).transpose(1, 0, 2).reshape(C, C)
    wp = np.asarray(inputs["Wproj"], dtype=f)
    w1 = np.asarray(inputs["W1"], dtype=f)
    w2 = np.asarray(inputs["W2"], dtype=f)
    b1 = np.asarray(inputs["b1"], dtype=f)

    common = {
        "wq": _pack_dr(g1[:, None] * wq),
        "wk": _pack_dr(g1[:, None] * wk),
        "wv": _pack_dr(g1[:, None] * wv),
        "wp": np.ascontiguousarray(wp).astype(BF16NP),
        "w1": _pack_dr(g2[:, None] * w1),
        "w2": np.ascontiguousarray(w2).astype(BF16NP),
        "colq": QS * (be1 @ wq),
        "colk": QS * (be1 @ wk),
        "b1c": HS * (b1 + be2 @ w1),
        "bvr": (WS * (be1 @ wv)).astype(BF16NP),
        "bp": np.asarray(inputs["bproj"], dtype=f),
        "b2": np.asarray(inputs["b2"], dtype=f),
    }
    return [{"x": x[b], **common} for b in range(N_CORES)]


def kernel(**inputs) -> np.ndarray:
    if "nc" not in _CACHE:
        _CACHE["nc"] = build_nc()
    nc = _CACHE["nc"]
    in_maps = _prep_inputs(inputs)
    res = run_bass_kernel_spmd(nc, in_maps, list(range(N_CORES)))
    out = np.stack([res.results[b]["out"] for b in range(N_CORES)], axis=0)
    return out.astype(np.float32)


if __name__ == "__main__":
    rng = np.random.default_rng(0)
    demo = {
        "x": rng.standard_normal((B, T, C), dtype=np.float32),
        "Wq": rng.standard_normal((H, C, Dh), dtype=np.float32) * 0.02,
        "Wk": rng.standard_normal((H, C, Dh), dtype=np.float32) * 0.02,
        "Wv": rng.standard_normal((H, C, Dh), dtype=np.float32) * 0.02,
        "Wproj": rng.standard_normal((C, C), dtype=np.float32) * 0.02,
        "bproj": np.zeros(C, np.float32),
        "W1": rng.standard_normal((C, C), dtype=np.float32) * 0.02,
        "b1": np.zeros(C, np.float32),
        "W2": rng.standard_normal((C, C), dtype=np.float32) * 0.02,
        "b2": np.zeros(C, np.float32),
        "g1": np.ones(C, np.float32),
        "beta1": np.zeros(C, np.float32),
        "g2": np.ones(C, np.float32),
        "beta2": np.zeros(C, np.float32),
    }
    y = kernel(**demo)
    print("out", y.shape, y.dtype, float(np.abs(y).max()))


# revision 50
# speedup vs baseline: 1.8935x; 1.0107x over previous
"""Trainium2 Bass kernel for nn_Block_3616362463321 (dense transformer block).

B=8, T=1024, C=1024, H=16, Dh=64. Data-parallel over batch: core b gets x[b].
Weights replicated to all 8 cores; no collectives.

v2 design (vs baseline):
  - All six weight GEMMs (QKV / proj / fc1 / fc2) run fp8e4m3 with
    perf_mode=DoubleRow: K=256 contraction per pass, 0.5 cycles/row.
    Weights are host-quantized (x1024, clip +-240) into an interleaved
    [q, p, islot, m] layout; activations are quantized on the fly into
    "mega" SBUF tiles [128, 8*1024] fp8 whose (k=c-chunk, t) layout serves
    both the DoubleRow moving-operand view [p, 2, t] and the stationary
    view [p, 2, 128].
  - LayerNorm statistics AND normalization happen in natural [t, c] layout
    (per-partition mean/rstd -> one tensor_scalar), with gamma folded into
    the weights host-side and beta folded into per-output bias columns.
    No DRAM stat bounces.  Normalized activations are PE-transposed as fp8
    (1 cycle/row), 4 blocks batched per PSUM bank.
  - proj and fc2 run bf16 (plain matmuls) - fp8 weights there cost too much
    accuracy on the direct output paths; V/attnT/E/y stay bf16 for the same
    reason.  Q/K/V/fc1 keep fp8 DoubleRow.
  - Attention keeps the baseline S^T orientation (fp8 Q/K, bf16 E, bf16 V
    with a fused ones column for softmax denominators), with:
      * S row-tile pairs (K=64 at rows 0/64) writing one 2-bank PSUM pair
        tile -> a single paired exp per (m, tn, i) on ACT,
      * exact causal trims everywhere (bf16/fp8 matmuls have no N>=256
        restriction),
      * causal masking as a post-exp bf16 triangle multiply (2x DVE mode),
      * pa tiles evacuated to SBUF immediately (frees PSUM banks from the
        denominator-bounce latency); sums rows reshaped to [128, 8] via DMA
        so the bit-exact DVE reciprocal runs lane-parallel (a [1,512]
        reciprocal is ~6 cycles/element serial), then a DRAM bounce
        broadcasts 1/sums; the normalize multiplies run on GpSimd.
  - fc2 residual fused via scalar_tensor_tensor; SBUF-only elementwise work
    (x+bias rows, y+b2, normalize mults) offloaded to the idle GpSimd
    engine; x/out rows on the SP DMA queue, weights on Pool/ACT queues.
"""
import sys

sys.path.insert(0, "/opt/trn_rl_repo")

from contextlib import ExitStack, nullcontext

import numpy as np
import ml_dtypes

import concourse.bacc as bacc
import concourse.bass as bass
import concourse.mybir as mybir
import concourse.tile as tile
from concourse.bass_utils import run_bass_kernel_spmd

P = 128
B, T, C, H = 8, 1024, 1024, 16
Dh = C // H            # 64
EPS = 1e-5
NF = 512               # matmul moving free dim (fp32 PSUM bank limit)
KC = C // P            # 8 c-chunks of 128
QC = C // 256          # 4 c-chunks of 256 (DoubleRow)
TJ = T // P            # 8 t-chunks of 128
TN = T // NF           # 2 t-chunks of 512
F32 = mybir.dt.float32
F32R = mybir.dt.float32r
BF16 = mybir.dt.bfloat16
F8 = mybir.dt.float8e4
ALU = mybir.AluOpType
ACTF = mybir.ActivationFunctionType
DR = mybir.MatmulPerfMode.DoubleRow

WS = 1024.0            # host weight upscale (fp8 range use)
QS = 8.0               # Q/K storage scale
SEXP = (Dh ** -0.5) / (QS * QS)   # exp scale absorbing Q/K storage scales
VIS = 1.0              # V ones-column value (bf16 attnT: true scale)
HS = 8.0               # hT storage scale

N_CORES = 8

_CACHE = {}

F8NP = ml_dtypes.float8_e4m3
BF16NP = ml_dtypes.bfloat16


def _bcast_row_ap(handle_ap, parts):
    """AP reading a [N]-shaped DRAM tensor broadcast across `parts` partitions."""
    return bass.AP(
        tensor=handle_ap.tensor,
        offset=handle_ap.offset,
        ap=[[0, parts], *handle_ap.ap],
    )


def build_nc(loop=1, hwloop=0, phases=7):
    nc = bacc.Bacc("TRN2", target_bir_lowering=False, debug=False)

    x_d = nc.dram_tensor("x", [T, C], F32, kind="ExternalInput")
    w_d = {}
    for nm in ("wq", "wk", "wv", "w1"):
        w_d[nm] = nc.dram_tensor(nm, [QC * P, 2 * C], F8, kind="ExternalInput")
    wp_d = nc.dram_tensor("wp", [C, C], BF16, kind="ExternalInput")
    w2_d = nc.dram_tensor("w2", [C, C], BF16, kind="ExternalInput")
    # bias columns pre-shaped host-side to [P, KC] (contiguous DMA; the
    # strided "(k p) -> p k" load costs 1024 4-byte descriptors)
    colq_d = nc.dram_tensor("colq", [P, KC], F32, kind="ExternalInput")
    colk_d = nc.dram_tensor("colk", [P, KC], F32, kind="ExternalInput")
    b1c_d = nc.dram_tensor("b1c", [P, KC], F32, kind="ExternalInput")
    bvr_d = nc.dram_tensor("bvr", [C], BF16, kind="ExternalInput")
    bp_d = nc.dram_tensor("bp", [C], F32, kind="ExternalInput")
    b2_d = nc.dram_tensor("b2", [C], F32, kind="ExternalInput")
    out_d = nc.dram_tensor("out", [T, C], F32, kind="ExternalOutput")

    identb_c = nc.inline_tensor(np.eye(P).astype(BF16NP), name="identb_c")
    # post-exp causal keep-mask for a diagonal [s,t] block: keep where s <= t
    tri_np = (np.arange(P)[:, None] <= np.arange(P)[None, :]).astype(BF16NP)
    tri_c = nc.inline_tensor(tri_np, name="tri_c")

    with tile.TileContext(nc) as tc, ExitStack() as ES:
        singles = ES.enter_context(tc.tile_pool(name="singles", bufs=1))
        dram = ES.enter_context(tc.tile_pool(name="drsc", bufs=1, space="DRAM"))

        identb = singles.tile([P, P], BF16)
        nc.sync.dma_start(out=identb[:], in_=identb_c.ap())
        tri01 = singles.tile([P, P], BF16)
        nc.sync.dma_start(out=tri01[:], in_=tri_c.ap())
        epsc = singles.tile([P, 1], F32)
        nc.vector.memset(epsc[:], EPS)
        onesb = singles.tile([1, P], BF16)
        nc.vector.memset(onesb[:], 1.0)

        # bias columns [P, KC]: element (p, k) = vec[k*P + p]
        cols = {}

        def load_col(nm, hd):
            t_ = singles.tile([P, KC], F32, tag=f"col_{nm}", name=f"col_{nm}")
            nc.gpsimd.dma_start(out=t_[:], in_=hd[:, :])
            cols[nm] = t_

        load_col("q", colq_d)
        load_col("k", colk_d)
        load_col("b1", b1c_d)
        bvr = singles.tile([1, C], BF16)
        nc.gpsimd.dma_start(out=bvr[:], in_=bvr_d.ap()[None, :])
        bpb = singles.tile([P, C], F32)
        nc.gpsimd.dma_start(out=bpb[:], in_=_bcast_row_ap(bp_d.ap(), P))
        b2b = singles.tile([P, C], F32)
        nc.gpsimd.dma_start(out=b2b[:], in_=_bcast_row_ap(b2_d.ap(), P))

        # ---- SBUF arena ----
        arena = ES.enter_context(tc.tile_pool(name="arena", bufs=1))

        def mega(tag):
            return arena.tile([P, KC * T], F8, tag=tag, name=tag)

        # weight tiles: [P, 2, C] fp8 per 256-chunk
        def wtiles(nm, share=None):
            tg = share or nm
            return [arena.tile([P, 2, C], F8, tag=f"{tg}_{q}", name=f"{nm}{q}")
                    for q in range(QC)]

        # ---- PSUM pool: declare SP pair tags first (2 banks each), then PA ----
        psum = ES.enter_context(tc.tile_pool(name="psum", bufs=1, space="PSUM"))
        _sp = [0]
        _pa = [0]

        def sptile(shape, dtype, nm="sp"):
            t = psum.tile(list(shape), dtype, tag=f"SP{_sp[0] % 2}",
                          name=f"{nm}{_sp[0]}")
            _sp[0] += 1
            return t

        def patile(shape=(P, NF), nm="pa"):
            t = psum.tile(list(shape), F32, tag=f"PA{_pa[0] % 4}",
                          name=f"{nm}{_pa[0]}")
            _pa[0] += 1
            return t

        # force tag declaration order: SP0, SP1 as [P, 2, NF] f32 (2 banks each)
        _ = psum.tile([P, 2, NF], F32, tag="SP0", name="spdecl0")
        _ = psum.tile([P, 2, NF], F32, tag="SP1", name="spdecl1")

        # weights on the Pool/ACT queues; x and out rows keep SP (+ACT) free-ish
        _dq = [0]
        _dqe = [nc.gpsimd, nc.scalar]

        def bulk_dma(out, in_):
            eng = _dqe[_dq[0] % len(_dqe)]
            _dq[0] += 1
            eng.dma_start(out=out, in_=in_)

        def row_dma(out, in_):
            nc.sync.dma_start(out=out, in_=in_)

        def ln_pass(src_tiles_or_loader, xn_tag, dst_mega, ph, out_rows_dtype=F8):
            """Natural-layout LN: per 128-row chunk j, bn_stats -> mean/rstd
            columns -> one tensor_scalar into an fp8 row tile -> PE-transpose
            (fp8, 4 blocks per PSUM batch) into dst_mega [(k t)] layout."""
            with ExitStack() as S:
                stp = S.enter_context(tc.tile_pool(name=f"stp{ph}", bufs=4))
                xnp = S.enter_context(tc.tile_pool(name=f"xnp{ph}", bufs=3))
                dv = dst_mega[:].rearrange("p (k t) -> p k t", k=KC)
                for j in range(TJ):
                    xj = src_tiles_or_loader(j)
                    st = stp.tile([P, 2, 6], F32, tag="st")
                    xr2 = xj[:].rearrange("p (g f) -> p g f", f=NF)
                    for g in range(2):
                        nc.vector.bn_stats(out=st[:, g, :], in_=xr2[:, g, :])
                    mv = stp.tile([P, 2], F32, tag="mv")
                    nc.vector.bn_aggr(out=mv[:], in_=st[:])
                    srt = stp.tile([P, 1], F32, tag="srt")
                    nc.scalar.activation(out=srt[:], in_=mv[:, 1:2],
                                         func=ACTF.Sqrt, bias=epsc[:], scale=1.0)
                    rc = stp.tile([P, 1], F32, tag="rc")
                    nc.vector.reciprocal(rc[:], srt[:])
                    xnr = xnp.tile([P, C], BF16, tag="xnr")
                    nc.vector.tensor_scalar(
                        out=xnr[:], in0=xj[:], scalar1=mv[:, 0:1], scalar2=rc[:],
                        op0=ALU.subtract, op1=ALU.mult)
                    for kb in range(2):
                        if kb == 0:
                            pt = sptile([P, 4 * P], BF16, "pt")
                        else:
                            pt = psum.tile([P, 4 * P], BF16,
                                           tag=f"PA{_pa[0] % 4}", name="ptb")
                            _pa[0] += 1
                        for k4 in range(4):
                            k = kb * 4 + k4
                            nc.tensor.transpose(pt[:, k4 * P:(k4 + 1) * P],
                                                xnr[:, k * P:(k + 1) * P],
                                                identb[:])
                        dst = dv[:, kb * 4:(kb + 1) * 4, j * P:(j + 1) * P]
                        src = pt[:].rearrange("p (a b) -> p a b", a=4)
                        if kb == 0:
                            nc.vector.tensor_copy(out=dst, in_=src)
                        else:
                            nc.scalar.activation(out=dst, in_=src,
                                                 func=ACTF.Copy)

        xrp = ES.enter_context(tc.tile_pool(name="xrp", bufs=3))

        def load_x(j):
            xj = xrp.tile([P, C], F32, tag="xrow")
            row_dma(out=xj[:], in_=x_d[j * P:(j + 1) * P, :])
            return xj

        with (tc.For_i(0, hwloop, 1) if hwloop else nullcontext()):
            for _it in range(loop):
                # ---------- weights: issue all DMAs up front (prefetch) ----------
                wq_sb = wtiles("wq")
                wk_sb = wtiles("wk")
                wv_sb = wtiles("wv")
                # w1/w2 reuse wv/wq slots (dead after QKV); DMAs self-order
                w1_sb = wtiles("w1", share="wv")
                w2_sb = [arena.tile([P, C], BF16,
                                    tag=(f"wq_{k}" if k < QC else f"wk_{k - QC}"),
                                    name=f"w2{k}") for k in range(KC)]
                for q in range(QC):
                    for nm, tl in (("wq", wq_sb), ("wk", wk_sb), ("wv", wv_sb)):
                        bulk_dma(out=tl[q][:], in_=w_d[nm][q * P:(q + 1) * P, :])
                # proj weights: plain bf16 [P, C] per c'-chunk
                wp_sb = [arena.tile([P, C], BF16, tag=f"wp_{k}", name=f"wp{k}")
                         for k in range(KC)]
                for k in range(KC):
                    bulk_dma(out=wp_sb[k][:], in_=wp_d[k * P:(k + 1) * P, :])

                # ---------- Phase 0: LN1 (stats + normalize + transpose) ----------
                xn8 = mega("XN1")
                ln_pass(load_x, "xn", xn8, 0)
                xnv = xn8[:].rearrange("p (k t) -> p k t", k=KC)

                if phases >= 2:
                    # ---------------- Phase 1: QKV (fp8 DoubleRow) ----------------
                    QT = [arena.tile([P, T], F8, tag=f"QT_{m}", name=f"QT{m}")
                          for m in range(KC)]
                    KTt = [arena.tile([P, T], F8, tag=f"KT_{m}", name=f"KT{m}")
                           for m in range(KC)]
                    for tn in range(TN):
                        tsl = slice(tn * NF, (tn + 1) * NF)
                        for m in range(KC):
                            pq = patile(nm="pq")
                            for q in range(QC):
                                nc.tensor.matmul(
                                    pq[:], lhsT=wq_sb[q][:, :, m * P:(m + 1) * P],
                                    rhs=xnv[:, 2 * q:2 * q + 2, tsl],
                                    start=(q == 0), stop=(q == QC - 1),
                                    perf_mode=DR)
                            nc.vector.tensor_scalar(
                                out=QT[m][:, tsl], in0=pq[:], scalar1=QS / WS,
                                scalar2=cols["q"][:, m:m + 1],
                                op0=ALU.mult, op1=ALU.add)
                            pk = patile(nm="pk")
                            for q in range(QC):
                                nc.tensor.matmul(
                                    pk[:], lhsT=wk_sb[q][:, :, m * P:(m + 1) * P],
                                    rhs=xnv[:, 2 * q:2 * q + 2, tsl],
                                    start=(q == 0), stop=(q == QC - 1),
                                    perf_mode=DR)
                            nc.vector.tensor_scalar(
                                out=KTt[m][:, tsl], in0=pk[:], scalar1=QS / WS,
                                scalar2=cols["k"][:, m:m + 1],
                                op0=ALU.mult, op1=ALU.add)

                    V = [arena.tile([P, H, Dh + 1], BF16, tag=f"V_{j}",
                                    name=f"V{j}") for j in range(TJ)]
                    for j in range(TJ):
                        nc.gpsimd.memset(V[j][:, :, Dh:Dh + 1], VIS)
                        for hn in range(TN):
                            hsl = slice(hn * NF, (hn + 1) * NF)
                            pv = patile(nm="pv")
                            nc.tensor.matmul(pv[:], lhsT=onesb[0:1, :],
                                             rhs=bvr[0:1, hsl],
                                             start=True, stop=False)
                            for q in range(QC):
                                nc.tensor.matmul(
                                    pv[:], lhsT=xnv[:, 2 * q:2 * q + 2,
                                                    j * P:(j + 1) * P],
                                    rhs=wv_sb[q][:, :, hsl],
                                    start=False, stop=(q == QC - 1),
                                    perf_mode=DR)
                            nc.scalar.activation(
                                out=V[j][:, hn * 8:(hn + 1) * 8, 0:Dh],
                                in_=pv[:].rearrange("p (h d) -> p h d", d=Dh),
                                func=ACTF.Identity, scale=1.0 / WS)
                    # late weights into the now-free wv/wq/wk slots
                    for q in range(QC):
                        bulk_dma(out=w1_sb[q][:],
                                 in_=w_d["w1"][q * P:(q + 1) * P, :])
                    for k in range(KC):
                        bulk_dma(out=w2_sb[k][:],
                                 in_=w2_d[k * P:(k + 1) * P, :])

                # proj residual rows (x + bproj) prefetched before attention
                # so proj can start the moment its attnT half is ready
                xrb = [arena.tile([P, C], BF16, tag=f"XRB_{j}", name=f"xrb{j}")
                       for j in range(TJ)]
                for j in range(TJ):
                    xj2 = load_x(j)
                    nc.gpsimd.tensor_tensor(xrb[j][:], xj2[:], bpb[:], ALU.add)

                attnT = arena.tile([P, KC * T], BF16, tag="ATT", name="attnT")
                atv = attnT[:].rearrange("p (m t) -> p m t", m=KC)
                if phases >= 3:
                    # ---------------- Phase 2: attention ----------------
                    with ExitStack() as S:
                        ep = S.enter_context(tc.tile_pool(name="ep", bufs=3))
                        rp = S.enter_context(tc.tile_pool(name="rp", bufs=3))
                        bp_ = S.enter_context(tc.tile_pool(name="bp", bufs=2))
                        tp1 = S.enter_context(tc.tile_pool(name="tp1", bufs=2))
                        for tn in range(TN):
                            tsl = slice(tn * NF, (tn + 1) * NF)
                            i_hi = 4 * (tn + 1)
                            for m in range(KC):
                                h0, h1 = 2 * m, 2 * m + 1
                                pa0 = patile((Dh + 1, NF), "pa0")
                                pa1 = patile((Dh + 1, NF), "pa1")
                                for i in range(i_hi):
                                    diag = i - 4 * tn
                                    d0 = max(diag, 0) * P
                                    esl = slice(d0, NF)
                                    qsl = slice(tn * NF + d0, (tn + 1) * NF)
                                    ssl = slice(i * P, (i + 1) * P)
                                    sp2 = sptile([P, 2, NF], F32, "s")
                                    nc.tensor.matmul(
                                        sp2[:, 0, esl], lhsT=KTt[m][0:64, ssl],
                                        rhs=QT[m][0:64, qsl],
                                        start=True, stop=True)
                                    nc.tensor.matmul(
                                        sp2[:, 1, esl], lhsT=KTt[m][64:128, ssl],
                                        rhs=QT[m][64:128, qsl],
                                        start=True, stop=True)
                                    Et = ep.tile([P, 2, NF], BF16, tag="E")
                                    nc.scalar.activation(
                                        out=Et[:, :, esl], in_=sp2[:, :, esl],
                                        func=ACTF.Exp, scale=SEXP)
                                    if diag >= 0:
                                        dsl = slice(d0, d0 + P)
                                        tri_b = bass.AP(
                                            tensor=tri01[:].tensor,
                                            offset=tri01[:].offset,
                                            ap=[tri01[:].ap[0], [0, 2],
                                                *tri01[:].ap[1:]])
                                        nc.vector.tensor_tensor(
                                            Et[:, :, dsl], Et[:, :, dsl],
                                            tri_b, ALU.mult)
                                    nc.tensor.matmul(
                                        pa0[:, esl], lhsT=V[i][:, h0, :],
                                        rhs=Et[:, 0, esl],
                                        start=(i == 0), stop=(i == i_hi - 1))
                                    nc.tensor.matmul(
                                        pa1[:, esl], lhsT=V[i][:, h1, :],
                                        rhs=Et[:, 1, esl],
                                        start=(i == 0), stop=(i == i_hi - 1))
                                # evacuate pa to SBUF immediately (frees the
                                # PSUM banks from the denominator-bounce
                                # latency), then: reshape sums to [128, 8] via
                                # DMA so the bit-exact reciprocal runs wide,
                                # bounce through DRAM for the broadcast, and
                                # normalize on the idle Pool engine.
                                av0 = rp.tile([Dh + 1, NF], F32, tag="av0")
                                av1 = rp.tile([Dh + 1, NF], F32, tag="av1")
                                nc.vector.tensor_copy(out=av0[:], in_=pa0[:])
                                nc.vector.tensor_copy(out=av1[:], in_=pa1[:])
                                s2 = rp.tile([P, 8], F32, tag="s2")
                                nc.gpsimd.dma_start(
                                    out=s2[:, 0:4],
                                    in_=av0[Dh:Dh + 1, :])
                                nc.gpsimd.dma_start(
                                    out=s2[:, 4:8],
                                    in_=av1[Dh:Dh + 1, :])
                                nc.vector.reciprocal(s2[:], s2[:])
                                drr = dram.tile([2 * NF], F32, tag="rsums")
                                nc.gpsimd.dma_start(
                                    out=drr[0:NF].rearrange("(p i) -> p i", i=4),
                                    in_=s2[:, 0:4])
                                nc.gpsimd.dma_start(
                                    out=drr[NF:2 * NF].rearrange(
                                        "(p i) -> p i", i=4),
                                    in_=s2[:, 4:8])
                                bct = bp_.tile([Dh, 2, NF], F32, tag="bct")
                                nc.sync.dma_start(
                                    out=bct[:, 0, :],
                                    in_=drr[0:NF][None, :].to_broadcast([Dh, NF]))
                                nc.sync.dma_start(
                                    out=bct[:, 1, :],
                                    in_=drr[NF:2 * NF][None, :].to_broadcast(
                                        [Dh, NF]))
                                nc.gpsimd.tensor_tensor(
                                    atv[0:Dh, m, tsl], av0[0:Dh, :],
                                    bct[:, 0, :], ALU.mult)
                                tmp1 = tp1.tile([Dh, NF], BF16, tag="t1")
                                nc.gpsimd.tensor_tensor(
                                    tmp1[:], av1[0:Dh, :], bct[:, 1, :], ALU.mult)
                                nc.gpsimd.dma_start(
                                    out=atv[Dh:2 * Dh, m, tsl], in_=tmp1[:])

                y_n = [arena.tile([P, C], BF16, tag=f"Y_{j}", name=f"y{j}")
                       for j in range(TJ)]
                if phases >= 4:
                    # ---------- Phase 3: proj + residual -> y (bf16, normal) -----
                    for j in range(TJ):
                        for nn in range(TN):
                            csl = slice(nn * NF, (nn + 1) * NF)
                            pp = patile(nm="pp")
                            for k in range(KC):
                                nc.tensor.matmul(
                                    pp[:], lhsT=atv[:, k, j * P:(j + 1) * P],
                                    rhs=wp_sb[k][:, csl],
                                    start=(k == 0), stop=(k == KC - 1))
                            nc.vector.tensor_tensor(
                                y_n[j][:, csl], pp[:], xrb[j][:, csl], ALU.add)

                if phases >= 5:
                    # ---------------- Phase 4: LN2 ----------------
                    xn28 = mega("XN2")
                    ln_pass(lambda j: y_n[j], "xn2", xn28, 1)
                    xn2v = xn28[:].rearrange("p (k t) -> p k t", k=KC)

                hT = arena.tile([P, KC * T], BF16, tag="HT", name="hT")
                htv = hT[:].rearrange("p (m t) -> p m t", m=KC)
                if phases >= 6:
                    # ---------------- Phase 5: MLP fc1 + relu ----------------
                    for tn in range(TN):
                        tsl = slice(tn * NF, (tn + 1) * NF)
                        for m in range(KC):
                            ph = patile(nm="ph")
                            for q in range(QC):
                                nc.tensor.matmul(
                                    ph[:], lhsT=w1_sb[q][:, :, m * P:(m + 1) * P],
                                    rhs=xn2v[:, 2 * q:2 * q + 2, tsl],
                                    start=(q == 0), stop=(q == QC - 1),
                                    perf_mode=DR)
                            nc.scalar.activation(
                                out=htv[:, m, tsl], in_=ph[:], func=ACTF.Relu,
                                bias=cols["b1"][:, m:m + 1], scale=HS / WS)

                if phases >= 7:
                    # ---------- Phase 6: MLP fc2 + residual -> out ----------
                    with ExitStack() as S:
                        otp = S.enter_context(tc.tile_pool(name="otp", bufs=3))
                        y2p = S.enter_context(tc.tile_pool(name="y2p", bufs=2))
                        for j in range(TJ):
                            y2 = y2p.tile([P, C], BF16, tag="y2")
                            nc.gpsimd.tensor_tensor(y2[:], y_n[j][:], b2b[:],
                                                    ALU.add)
                            for nn in range(TN):
                                csl = slice(nn * NF, (nn + 1) * NF)
                                po = patile(nm="po")
                                for k in range(KC):
                                    nc.tensor.matmul(
                                        po[:], lhsT=htv[:, k, j * P:(j + 1) * P],
                                        rhs=w2_sb[k][:, csl],
                                        start=(k == 0), stop=(k == KC - 1))
                                ot = otp.tile([P, NF], F32, tag="ot")
                                nc.vector.scalar_tensor_tensor(
                                    out=ot[:], in0=po[:], scalar=1.0 / HS,
                                    in1=y2[:, csl], op0=ALU.mult, op1=ALU.add)
                                bulk_dma(out=out_d[j * P:(j + 1) * P, csl],
                                         in_=ot[:])

    nc.compile()
    return nc


def _f8(a):
    return np.clip(a, -240.0, 240.0).astype(F8NP)


def _pack_dr(w_eff):
    """[C, M] effective weight -> [QC*P, 2*M] fp8 DoubleRow layout
    (row q*128+p, col i*M+m  <-  w_eff[q*256 + i*128 + p, m] * WS)."""
    M = w_eff.shape[1]
    w = (w_eff * WS).reshape(QC, 2, P, M).transpose(0, 2, 1, 3).reshape(
        QC * P, 2 * M)
    return _f8(np.ascontiguousarray(w))


def _prep_inputs(inputs):
    """Host-side weight repacking/quantization; returns per-core in_maps."""
    f = np.float32
    x = np.ascontiguousarray(np.asarray(inputs["x"], dtype=f))        # [B, T, C]
    g1 = np.asarray(inputs["g1"], dtype=f)
    be1 = np.asarray(inputs["beta1"], dtype=f)
    g2 = np.asarray(inputs["g2"], dtype=f)
    be2 = np.asarray(inputs["beta2"], dtype=f)

    wq = np.asarray(inputs["Wq"], dtype=f).transpose(1, 0, 2).reshape(C, C)
    wk = np.asarray(inputs["Wk"], dtype=f).transpose(1, 0, 2).reshape(C, C)
    wv = np.asarray(inputs["Wv"], dtype=f).transpose(1, 0, 2).reshape(C, C)
    wp = np.asarray(inputs["Wproj"], dtype=f)
    w1 = np.asarray(inputs["W1"], dtype=f)
    w2 = np.asarray(inputs["W2"], dtype=f)
    b1 = np.asarray(inputs["b1"], dtype=f)

    common = {
        "wq": _pack_dr(g1[:, None] * wq),
        "wk": _pack_dr(g1[:, None] * wk),
        "wv": _pack_dr(g1[:, None] * wv),
        "wp": np.ascontiguousarray(wp).astype(BF16NP),
        "w1": _pack_dr(g2[:, None] * w1),
        "w2": np.ascontiguousarray(w2).astype(BF16NP),
        "colq": np.ascontiguousarray(
            (QS * (be1 @ wq)).reshape(KC, P).T),
        "colk": np.ascontiguousarray(
            (QS * (be1 @ wk)).reshape(KC, P).T),
        "b1c": np.ascontiguousarray(
            (HS * (b1 + be2 @ w1)).reshape(KC, P).T),
        "bvr": (WS * (be1 @ wv)).astype(BF16NP),
        "bp": np.asarray(inputs["bproj"], dtype=f),
        "b2": np.asarray(inputs["b2"], dtype=f),
    }
    return [{"x": x[b], **common} for b in range(N_CORES)]


def kernel(**inputs) -> np.ndarray:
    if "nc" not in _CACHE:
        _CACHE["nc"] = build_nc()
    nc = _CACHE["nc"]
    in_maps = _prep_inputs(inputs)
    res = run_bass_kernel_spmd(nc, in_maps, list(range(N_CORES)))
    out = np.stack([res.results[b]["out"] for b in range(N_CORES)], axis=0)
    return out.astype(np.float32)


if __name__ == "__main__":
    rng = np.random.default_rng(0)
    demo = {
        "x": rng.standard_normal((B, T, C), dtype=np.float32),
        "Wq": rng.standard_normal((H, C, Dh), dtype=np.float32) * 0.02,
        "Wk": rng.standard_normal((H, C, Dh), dtype=np.float32) * 0.02,
        "Wv": rng.standard_normal((H, C, Dh), dtype=np.float32) * 0.02,
        "Wproj": rng.standard_normal((C, C), dtype=np.float32) * 0.02,
        "bproj": np.zeros(C, np.float32),
        "W1": rng.standard_normal((C, C), dtype=np.float32) * 0.02,
        "b1": np.zeros(C, np.float32),
        "W2": rng.standard_normal((C, C), dtype=np.float32) * 0.02,
        "b2": np.zeros(C, np.float32),
        "g1": np.ones(C, np.float32),
        "beta1": np.zeros(C, np.float32),
        "g2": np.ones(C, np.float32),
        "beta2": np.zeros(C, np.float32),
    }
    y = kernel(**demo)
    print("out", y.shape, y.dtype, float(np.abs(y).max()))


# revision 52
# speedup vs baseline: 1.9038x; 1.0055x over previous
"""Trainium2 Bass kernel for nn_Block_3616362463321 (dense transformer block).

B=8, T=1024, C=1024, H=16, Dh=64. Data-parallel over batch: core b gets x[b].
Weights replicated to all 8 cores; no collectives.

v2 design (vs baseline):
  - All six weight GEMMs (QKV / proj / fc1 / fc2) run fp8e4m3 with
    perf_mode=DoubleRow: K=256 contraction per pass, 0.5 cycles/row.
    Weights are host-quantized (x1024, clip +-240) into an interleaved
    [q, p, islot, m] layout; activations are quantized on the fly into
    "mega" SBUF tiles [128, 8*1024] fp8 whose (k=c-chunk, t) layout serves
    both the DoubleRow moving-operand view [p, 2, t] and the stationary
    view [p, 2, 128].
  - LayerNorm statistics AND normalization happen in natural [t, c] layout
    (per-partition mean/rstd -> one tensor_scalar), with gamma folded into
    the weights host-side and beta folded into per-output bias columns.
    No DRAM stat bounces.  Normalized activations are PE-transposed as fp8
    (1 cycle/row), 4 blocks batched per PSUM bank.
  - proj and fc2 run bf16 (plain matmuls) - fp8 weights there cost too much
    accuracy on the direct output paths; V/attnT/E/y stay bf16 for the same
    reason.  Q/K/V/fc1 keep fp8 DoubleRow.
  - Attention keeps the baseline S^T orientation (fp8 Q/K, bf16 E, bf16 V
    with a fused ones column for softmax denominators), with:
      * S row-tile pairs (K=64 at rows 0/64) writing one 2-bank PSUM pair
        tile -> a single paired exp per (m, tn, i) on ACT,
      * exact causal trims everywhere (bf16/fp8 matmuls have no N>=256
        restriction),
      * causal masking as a post-exp bf16 triangle multiply (2x DVE mode),
      * pa tiles evacuated to SBUF immediately (frees PSUM banks from the
        denominator-bounce latency); sums rows reshaped to [128, 8] via DMA
        so the bit-exact DVE reciprocal runs lane-parallel (a [1,512]
        reciprocal is ~6 cycles/element serial), then a DRAM bounce
        broadcasts 1/sums; the normalize multiplies run on GpSimd.
  - fc2 residual fused via scalar_tensor_tensor; SBUF-only elementwise work
    (x+bias rows, y+b2, normalize mults) offloaded to the idle GpSimd
    engine; x/out rows on the SP DMA queue, weights on Pool/ACT queues.
"""
import sys

sys.path.insert(0, "/opt/trn_rl_repo")

from contextlib import ExitStack, nullcontext

import numpy as np
import ml_dtypes

import concourse.bacc as bacc
import concourse.bass as bass
import concourse.mybir as mybir
import concourse.tile as tile
from concourse.bass_utils import run_bass_kernel_spmd

P = 128
B, T, C, H = 8, 1024, 1024, 16
Dh = C // H            # 64
EPS = 1e-5
NF = 512               # matmul moving free dim (fp32 PSUM bank limit)
KC = C // P            # 8 c-chunks of 128
QC = C // 256          # 4 c-chunks of 256 (DoubleRow)
TJ = T // P            # 8 t-chunks of 128
TN = T // NF           # 2 t-chunks of 512
F32 = mybir.dt.float32
F32R = mybir.dt.float32r
BF16 = mybir.dt.bfloat16
F8 = mybir.dt.float8e4
ALU = mybir.AluOpType
ACTF = mybir.ActivationFunctionType
DR = mybir.MatmulPerfMode.DoubleRow

WS = 1024.0            # host weight upscale (fp8 range use)
QS = 8.0               # Q/K storage scale
SEXP = (Dh ** -0.5) / (QS * QS)   # exp scale absorbing Q/K storage scales
VIS = 1.0              # V ones-column value (bf16 attnT: true scale)
HS = 8.0               # hT storage scale

N_CORES = 8

_CACHE = {}

F8NP = ml_dtypes.float8_e4m3
BF16NP = ml_dtypes.bfloat16


def _bcast_row_ap(handle_ap, parts):
    """AP reading a [N]-shaped DRAM tensor broadcast across `parts` partitions."""
    return bass.AP(
        tensor=handle_ap.tensor,
        offset=handle_ap.offset,
        ap=[[0, parts], *handle_ap.ap],
    )


def build_nc(loop=1, hwloop=0, phases=7):
    nc = bacc.Bacc("TRN2", target_bir_lowering=False, debug=False)

    x_d = nc.dram_tensor("x", [T, C], F32, kind="ExternalInput")
    w_d = {}
    for nm in ("wq", "wk", "wv", "w1"):
        w_d[nm] = nc.dram_tensor(nm, [QC * P, 2 * C], F8, kind="ExternalInput")
    wp_d = nc.dram_tensor("wp", [C, C], BF16, kind="ExternalInput")
    w2_d = nc.dram_tensor("w2", [C, C], BF16, kind="ExternalInput")
    # bias columns pre-shaped host-side to [P, KC] (contiguous DMA; the
    # strided "(k p) -> p k" load costs 1024 4-byte descriptors)
    colq_d = nc.dram_tensor("colq", [P, KC], F32, kind="ExternalInput")
    colk_d = nc.dram_tensor("colk", [P, KC], F32, kind="ExternalInput")
    b1c_d = nc.dram_tensor("b1c", [P, KC], F32, kind="ExternalInput")
    bvr_d = nc.dram_tensor("bvr", [C], BF16, kind="ExternalInput")
    bp_d = nc.dram_tensor("bp", [C], F32, kind="ExternalInput")
    b2_d = nc.dram_tensor("b2", [C], F32, kind="ExternalInput")
    out_d = nc.dram_tensor("out", [T, C], F32, kind="ExternalOutput")

    identb_c = nc.inline_tensor(np.eye(P).astype(BF16NP), name="identb_c")
    # post-exp causal keep-mask for a diagonal [s,t] block: keep where s <= t
    tri_np = (np.arange(P)[:, None] <= np.arange(P)[None, :]).astype(BF16NP)
    tri_c = nc.inline_tensor(tri_np, name="tri_c")

    with tile.TileContext(nc) as tc, ExitStack() as ES:
        singles = ES.enter_context(tc.tile_pool(name="singles", bufs=1))
        dram = ES.enter_context(tc.tile_pool(name="drsc", bufs=1, space="DRAM"))

        identb = singles.tile([P, P], BF16)
        nc.sync.dma_start(out=identb[:], in_=identb_c.ap())
        tri01 = singles.tile([P, P], BF16)
        nc.sync.dma_start(out=tri01[:], in_=tri_c.ap())
        epsc = singles.tile([P, 1], F32)
        nc.vector.memset(epsc[:], EPS)
        onesb = singles.tile([1, P], BF16)
        nc.vector.memset(onesb[:], 1.0)

        # bias columns [P, KC]: element (p, k) = vec[k*P + p]
        cols = {}

        def load_col(nm, hd):
            t_ = singles.tile([P, KC], F32, tag=f"col_{nm}", name=f"col_{nm}")
            nc.gpsimd.dma_start(out=t_[:], in_=hd[:, :])
            cols[nm] = t_

        load_col("q", colq_d)
        load_col("k", colk_d)
        load_col("b1", b1c_d)
        bvr = singles.tile([1, C], BF16)
        nc.gpsimd.dma_start(out=bvr[:], in_=bvr_d.ap()[None, :])
        bpb = singles.tile([P, C], F32)
        nc.gpsimd.dma_start(out=bpb[:], in_=_bcast_row_ap(bp_d.ap(), P))
        b2b = singles.tile([P, C], F32)
        nc.gpsimd.dma_start(out=b2b[:], in_=_bcast_row_ap(b2_d.ap(), P))

        # ---- SBUF arena ----
        arena = ES.enter_context(tc.tile_pool(name="arena", bufs=1))

        def mega(tag):
            return arena.tile([P, KC * T], F8, tag=tag, name=tag)

        # weight tiles: [P, 2, C] fp8 per 256-chunk
        def wtiles(nm, share=None):
            tg = share or nm
            return [arena.tile([P, 2, C], F8, tag=f"{tg}_{q}", name=f"{nm}{q}")
                    for q in range(QC)]

        # ---- PSUM pool: declare SP pair tags first (2 banks each), then PA ----
        psum = ES.enter_context(tc.tile_pool(name="psum", bufs=1, space="PSUM"))
        _sp = [0]
        _pa = [0]

        def sptile(shape, dtype, nm="sp"):
            t = psum.tile(list(shape), dtype, tag=f"SP{_sp[0] % 2}",
                          name=f"{nm}{_sp[0]}")
            _sp[0] += 1
            return t

        def patile(shape=(P, NF), nm="pa"):
            t = psum.tile(list(shape), F32, tag=f"PA{_pa[0] % 4}",
                          name=f"{nm}{_pa[0]}")
            _pa[0] += 1
            return t

        # force tag declaration order: SP0, SP1 as [P, 2, NF] f32 (2 banks each)
        _ = psum.tile([P, 2, NF], F32, tag="SP0", name="spdecl0")
        _ = psum.tile([P, 2, NF], F32, tag="SP1", name="spdecl1")

        # weights on the Pool/ACT queues; x and out rows keep SP (+ACT) free-ish
        _dq = [0]
        _dqe = [nc.gpsimd, nc.scalar]

        def bulk_dma(out, in_):
            eng = _dqe[_dq[0] % len(_dqe)]
            _dq[0] += 1
            eng.dma_start(out=out, in_=in_)

        def row_dma(out, in_):
            nc.sync.dma_start(out=out, in_=in_)

        def ln_pass(src_tiles_or_loader, xn_tag, dst_mega, ph, out_rows_dtype=F8):
            """Natural-layout LN: per 128-row chunk j, bn_stats -> mean/rstd
            columns -> one tensor_scalar into an fp8 row tile -> PE-transpose
            (fp8, 4 blocks per PSUM batch) into dst_mega [(k t)] layout."""
            with ExitStack() as S:
                stp = S.enter_context(tc.tile_pool(name=f"stp{ph}", bufs=4))
                xnp = S.enter_context(tc.tile_pool(name=f"xnp{ph}", bufs=3))
                dv = dst_mega[:].rearrange("p (k t) -> p k t", k=KC)
                for j in range(TJ):
                    xj = src_tiles_or_loader(j)
                    st = stp.tile([P, 2, 6], F32, tag="st")
                    xr2 = xj[:].rearrange("p (g f) -> p g f", f=NF)
                    for g in range(2):
                        nc.vector.bn_stats(out=st[:, g, :], in_=xr2[:, g, :])
                    mv = stp.tile([P, 2], F32, tag="mv")
                    nc.vector.bn_aggr(out=mv[:], in_=st[:])
                    srt = stp.tile([P, 1], F32, tag="srt")
                    nc.scalar.activation(out=srt[:], in_=mv[:, 1:2],
                                         func=ACTF.Sqrt, bias=epsc[:], scale=1.0)
                    rc = stp.tile([P, 1], F32, tag="rc")
                    nc.vector.reciprocal(rc[:], srt[:])
                    xnr = xnp.tile([P, C], BF16, tag="xnr")
                    nc.vector.tensor_scalar(
                        out=xnr[:], in0=xj[:], scalar1=mv[:, 0:1], scalar2=rc[:],
                        op0=ALU.subtract, op1=ALU.mult)
                    for kb in range(2):
                        if kb == 0:
                            pt = sptile([P, 4 * P], BF16, "pt")
                        else:
                            pt = psum.tile([P, 4 * P], BF16,
                                           tag=f"PA{_pa[0] % 4}", name="ptb")
                            _pa[0] += 1
                        for k4 in range(4):
                            k = kb * 4 + k4
                            nc.tensor.transpose(pt[:, k4 * P:(k4 + 1) * P],
                                                xnr[:, k * P:(k + 1) * P],
                                                identb[:])
                        dst = dv[:, kb * 4:(kb + 1) * 4, j * P:(j + 1) * P]
                        src = pt[:].rearrange("p (a b) -> p a b", a=4)
                        if kb == 0:
                            nc.vector.tensor_copy(out=dst, in_=src)
                        else:
                            nc.scalar.activation(out=dst, in_=src,
                                                 func=ACTF.Copy)

        xrp = ES.enter_context(tc.tile_pool(name="xrp", bufs=3))

        def load_x(j):
            xj = xrp.tile([P, C], F32, tag="xrow")
            row_dma(out=xj[:], in_=x_d[j * P:(j + 1) * P, :])
            return xj

        with (tc.For_i(0, hwloop, 1) if hwloop else nullcontext()):
            for _it in range(loop):
                # ---------- weights: issue all DMAs up front (prefetch) ----------
                wq_sb = wtiles("wq")
                wk_sb = wtiles("wk")
                wv_sb = wtiles("wv")
                # w1/w2 reuse wv/wq slots (dead after QKV); DMAs self-order
                w1_sb = wtiles("w1", share="wv")
                w2_sb = [arena.tile([P, C], BF16,
                                    tag=(f"wq_{k}" if k < QC else f"wk_{k - QC}"),
                                    name=f"w2{k}") for k in range(KC)]
                for q in range(QC):
                    for nm, tl in (("wq", wq_sb), ("wk", wk_sb), ("wv", wv_sb)):
                        bulk_dma(out=tl[q][:], in_=w_d[nm][q * P:(q + 1) * P, :])
                # proj weights: plain bf16 [P, C] per c'-chunk
                wp_sb = [arena.tile([P, C], BF16, tag=f"wp_{k}", name=f"wp{k}")
                         for k in range(KC)]
                for k in range(KC):
                    bulk_dma(out=wp_sb[k][:], in_=wp_d[k * P:(k + 1) * P, :])

                # ---------- Phase 0: LN1 (stats + normalize + transpose) ----------
                xn8 = mega("XN1")
                ln_pass(load_x, "xn", xn8, 0)
                xnv = xn8[:].rearrange("p (k t) -> p k t", k=KC)

                if phases >= 2:
                    # ---------------- Phase 1: QKV (fp8 DoubleRow) ----------------
                    QT = [arena.tile([P, T], F8, tag=f"QT_{m}", name=f"QT{m}")
                          for m in range(KC)]
                    KTt = [arena.tile([P, T], F8, tag=f"KT_{m}", name=f"KT{m}")
                           for m in range(KC)]
                    for tn in range(TN):
                        tsl = slice(tn * NF, (tn + 1) * NF)
                        for m in range(KC):
                            pq = patile(nm="pq")
                            for q in range(QC):
                                nc.tensor.matmul(
                                    pq[:], lhsT=wq_sb[q][:, :, m * P:(m + 1) * P],
                                    rhs=xnv[:, 2 * q:2 * q + 2, tsl],
                                    start=(q == 0), stop=(q == QC - 1),
                                    perf_mode=DR)
                            nc.vector.tensor_scalar(
                                out=QT[m][:, tsl], in0=pq[:], scalar1=QS / WS,
                                scalar2=cols["q"][:, m:m + 1],
                                op0=ALU.mult, op1=ALU.add)
                            pk = patile(nm="pk")
                            for q in range(QC):
                                nc.tensor.matmul(
                                    pk[:], lhsT=wk_sb[q][:, :, m * P:(m + 1) * P],
                                    rhs=xnv[:, 2 * q:2 * q + 2, tsl],
                                    start=(q == 0), stop=(q == QC - 1),
                                    perf_mode=DR)
                            nc.vector.tensor_scalar(
                                out=KTt[m][:, tsl], in0=pk[:], scalar1=QS / WS,
                                scalar2=cols["k"][:, m:m + 1],
                                op0=ALU.mult, op1=ALU.add)

                    V = [arena.tile([P, H, Dh + 1], BF16, tag=f"V_{j}",
                                    name=f"V{j}") for j in range(TJ)]
                    for j in range(TJ):
                        nc.gpsimd.memset(V[j][:, :, Dh:Dh + 1], VIS)
                        for hn in range(TN):
                            hsl = slice(hn * NF, (hn + 1) * NF)
                            pv = patile(nm="pv")
                            nc.tensor.matmul(pv[:], lhsT=onesb[0:1, :],
                                             rhs=bvr[0:1, hsl],
                                             start=True, stop=False)
                            for q in range(QC):
                                nc.tensor.matmul(
                                    pv[:], lhsT=xnv[:, 2 * q:2 * q + 2,
                                                    j * P:(j + 1) * P],
                                    rhs=wv_sb[q][:, :, hsl],
                                    start=False, stop=(q == QC - 1),
                                    perf_mode=DR)
                            nc.scalar.activation(
                                out=V[j][:, hn * 8:(hn + 1) * 8, 0:Dh],
                                in_=pv[:].rearrange("p (h d) -> p h d", d=Dh),
                                func=ACTF.Identity, scale=1.0 / WS)
                    # late weights into the now-free wv/wq/wk slots
                    for q in range(QC):
                        bulk_dma(out=w1_sb[q][:],
                                 in_=w_d["w1"][q * P:(q + 1) * P, :])
                    for k in range(KC):
                        bulk_dma(out=w2_sb[k][:],
                                 in_=w2_d[k * P:(k + 1) * P, :])

                # proj residual rows (x + bproj) prefetched before attention
                # so proj can start the moment its attnT half is ready
                xrb = [arena.tile([P, C], BF16, tag=f"XRB_{j}", name=f"xrb{j}")
                       for j in range(TJ)]
                for j in range(TJ):
                    xj2 = load_x(j)
                    nc.gpsimd.tensor_tensor(xrb[j][:], xj2[:], bpb[:], ALU.add)

                attnT = arena.tile([P, KC * T], BF16, tag="ATT", name="attnT")
                atv = attnT[:].rearrange("p (m t) -> p m t", m=KC)
                if phases >= 3:
                    # ---------------- Phase 2: attention ----------------
                    with ExitStack() as S:
                        ep = S.enter_context(tc.tile_pool(name="ep", bufs=3))
                        rp = S.enter_context(tc.tile_pool(name="rp", bufs=4))
                        bp_ = S.enter_context(tc.tile_pool(name="bp", bufs=3))
                        tp1 = S.enter_context(tc.tile_pool(name="tp1", bufs=3))
                        for tn in range(TN):
                            tsl = slice(tn * NF, (tn + 1) * NF)
                            i_hi = 4 * (tn + 1)
                            for m in range(KC):
                                h0, h1 = 2 * m, 2 * m + 1
                                pa0 = patile((Dh + 1, NF), "pa0")
                                pa1 = patile((Dh + 1, NF), "pa1")
                                for i in range(i_hi):
                                    diag = i - 4 * tn
                                    d0 = max(diag, 0) * P
                                    esl = slice(d0, NF)
                                    qsl = slice(tn * NF + d0, (tn + 1) * NF)
                                    ssl = slice(i * P, (i + 1) * P)
                                    sp2 = sptile([P, 2, NF], F32, "s")
                                    nc.tensor.matmul(
                                        sp2[:, 0, esl], lhsT=KTt[m][0:64, ssl],
                                        rhs=QT[m][0:64, qsl],
                                        start=True, stop=True)
                                    nc.tensor.matmul(
                                        sp2[:, 1, esl], lhsT=KTt[m][64:128, ssl],
                                        rhs=QT[m][64:128, qsl],
                                        start=True, stop=True)
                                    Et = ep.tile([P, 2, NF], BF16, tag="E")
                                    nc.scalar.activation(
                                        out=Et[:, :, esl], in_=sp2[:, :, esl],
                                        func=ACTF.Exp, scale=SEXP)
                                    if diag >= 0:
                                        dsl = slice(d0, d0 + P)
                                        tri_b = bass.AP(
                                            tensor=tri01[:].tensor,
                                            offset=tri01[:].offset,
                                            ap=[tri01[:].ap[0], [0, 2],
                                                *tri01[:].ap[1:]])
                                        nc.vector.tensor_tensor(
                                            Et[:, :, dsl], Et[:, :, dsl],
                                            tri_b, ALU.mult)
                                    nc.tensor.matmul(
                                        pa0[:, esl], lhsT=V[i][:, h0, :],
                                        rhs=Et[:, 0, esl],
                                        start=(i == 0), stop=(i == i_hi - 1))
                                    nc.tensor.matmul(
                                        pa1[:, esl], lhsT=V[i][:, h1, :],
                                        rhs=Et[:, 1, esl],
                                        start=(i == 0), stop=(i == i_hi - 1))
                                # evacuate pa to SBUF immediately (frees the
                                # PSUM banks from the denominator-bounce
                                # latency), then: reshape sums to [128, 8] via
                                # DMA so the bit-exact reciprocal runs wide,
                                # bounce through DRAM for the broadcast, and
                                # normalize on the idle Pool engine.
                                av0 = rp.tile([Dh + 1, NF], BF16, tag="av0")
                                av1 = rp.tile([Dh + 1, NF], BF16, tag="av1")
                                nc.vector.tensor_copy(out=av0[:], in_=pa0[:])
                                nc.vector.tensor_copy(out=av1[:], in_=pa1[:])
                                s2b = rp.tile([P, 8], BF16, tag="s2b")
                                nc.gpsimd.dma_start(
                                    out=s2b[:, 0:4],
                                    in_=av0[Dh:Dh + 1, :])
                                nc.gpsimd.dma_start(
                                    out=s2b[:, 4:8],
                                    in_=av1[Dh:Dh + 1, :])
                                s2 = rp.tile([P, 8], F32, tag="s2")
                                nc.vector.reciprocal(s2[:], s2b[:])
                                drr = dram.tile([2 * NF], F32, tag="rsums")
                                nc.gpsimd.dma_start(
                                    out=drr[0:NF].rearrange("(p i) -> p i", i=4),
                                    in_=s2[:, 0:4])
                                nc.gpsimd.dma_start(
                                    out=drr[NF:2 * NF].rearrange(
                                        "(p i) -> p i", i=4),
                                    in_=s2[:, 4:8])
                                bct = bp_.tile([Dh, 2, NF], F32, tag="bct")
                                nc.sync.dma_start(
                                    out=bct[:, 0, :],
                                    in_=drr[0:NF][None, :].to_broadcast([Dh, NF]))
                                nc.sync.dma_start(
                                    out=bct[:, 1, :],
                                    in_=drr[NF:2 * NF][None, :].to_broadcast(
                                        [Dh, NF]))
                                nc.gpsimd.tensor_tensor(
                                    atv[0:Dh, m, tsl], av0[0:Dh, :],
                                    bct[:, 0, :], ALU.mult)
                                tmp1 = tp1.tile([Dh, NF], BF16, tag="t1")
                                nc.gpsimd.tensor_tensor(
                                    tmp1[:], av1[0:Dh, :], bct[:, 1, :], ALU.mult)
                                nc.gpsimd.dma_start(
                                    out=atv[Dh:2 * Dh, m, tsl], in_=tmp1[:])

                y_n = [arena.tile([P, C], BF16, tag=f"Y_{j}", name=f"y{j}")
                       for j in range(TJ)]
                if phases >= 4:
                    # ---------- Phase 3: proj + residual -> y (bf16, normal) -----
                    for j in range(TJ):
                        for nn in range(TN):
                            csl = slice(nn * NF, (nn + 1) * NF)
                            pp = patile(nm="pp")
                            for k in range(KC):
                                nc.tensor.matmul(
                                    pp[:], lhsT=atv[:, k, j * P:(j + 1) * P],
                                    rhs=wp_sb[k][:, csl],
                                    start=(k == 0), stop=(k == KC - 1))
                            nc.vector.tensor_tensor(
                                y_n[j][:, csl], pp[:], xrb[j][:, csl], ALU.add)

                if phases >= 5:
                    # ---------------- Phase 4: LN2 ----------------
                    xn28 = mega("XN2")
                    ln_pass(lambda j: y_n[j], "xn2", xn28, 1)
                    xn2v = xn28[:].rearrange("p (k t) -> p k t", k=KC)

                hT = arena.tile([P, KC * T], BF16, tag="HT", name="hT")
                htv = hT[:].rearrange("p (m t) -> p m t", m=KC)
                if phases >= 6:
                    # ---------------- Phase 5: MLP fc1 + relu ----------------
                    for tn in range(TN):
                        tsl = slice(tn * NF, (tn + 1) * NF)
                        for m in range(KC):
                            ph = patile(nm="ph")
                            for q in range(QC):
                                nc.tensor.matmul(
                                    ph[:], lhsT=w1_sb[q][:, :, m * P:(m + 1) * P],
                                    rhs=xn2v[:, 2 * q:2 * q + 2, tsl],
                                    start=(q == 0), stop=(q == QC - 1),
                                    perf_mode=DR)
                            nc.scalar.activation(
                                out=htv[:, m, tsl], in_=ph[:], func=ACTF.Relu,
                                bias=cols["b1"][:, m:m + 1], scale=HS / WS)

                if phases >= 7:
                    # ---------- Phase 6: MLP fc2 + residual -> out ----------
                    with ExitStack() as S:
                        otp = S.enter_context(tc.tile_pool(name="otp", bufs=3))
                        y2p = S.enter_context(tc.tile_pool(name="y2p", bufs=2))
                        for j in range(TJ):
                            y2 = y2p.tile([P, C], BF16, tag="y2")
                            nc.gpsimd.tensor_tensor(y2[:], y_n[j][:], b2b[:],
                                                    ALU.add)
                            for nn in range(TN):
                                csl = slice(nn * NF, (nn + 1) * NF)
                                po = patile(nm="po")
                                for k in range(KC):
                                    nc.tensor.matmul(
                                        po[:], lhsT=htv[:, k, j * P:(j + 1) * P],
                                        rhs=w2_sb[k][:, csl],
                                        start=(k == 0), stop=(k == KC - 1))
                                ot = otp.tile([P, NF], F32, tag="ot")
                                nc.vector.scalar_tensor_tensor(
                                    out=ot[:], in0=po[:], scalar=1.0 / HS,
                                    in1=y2[:, csl], op0=ALU.mult, op1=ALU.add)
                                bulk_dma(out=out_d[j * P:(j + 1) * P, csl],
                                         in_=ot[:])

    nc.compile()
    return nc


def _f8(a):
    return np.clip(a, -240.0, 240.0).astype(F8NP)


def _pack_dr(w_eff):
    """[C, M] effective weight -> [QC*P, 2*M] fp8 DoubleRow layout
    (row q*128+p, col i*M+m  <-  w_eff[q*256 + i*128 + p, m] * WS)."""
    M = w_eff.shape[1]
    w = (w_eff * WS).reshape(QC, 2, P, M).transpose(0, 2, 1, 3).reshape(
        QC * P, 2 * M)
    return _f8(np.ascontiguousarray(w))


def _prep_inputs(inputs):
    """Host-side weight repacking/quantization; returns per-core in_maps."""
    f = np.float32
    x = np.ascontiguousarray(np.asarray(inputs["x"], dtype=f))        # [B, T, C]
    g1 = np.asarray(inputs["g1"], dtype=f)
    be1 = np.asarray(inputs["beta1"], dtype=f)
    g2 = np.asarray(inputs["g2"], dtype=f)
    be2 = np.asarray(inputs["beta2"], dtype=f)

    wq = np.asarray(inputs["Wq"], dtype=f).transpose(1, 0, 2).reshape(C, C)
    wk = np.asarray(inputs["Wk"], dtype=f).transpose(1, 0, 2).reshape(C, C)
    wv = np.asarray(inputs["Wv"], dtype=f).transpose(1, 0, 2).reshape(C, C)
    wp = np.asarray(inputs["Wproj"], dtype=f)
    w1 = np.asarray(inputs["W1"], dtype=f)
    w2 = np.asarray(inputs["W2"], dtype=f)
    b1 = np.asarray(inputs["b1"], dtype=f)

    common = {
        "wq": _pack_dr(g1[:, None] * wq),
        "wk": _pack_dr(g1[:, None] * wk),
        "wv": _pack_dr(g1[:, None] * wv),
        "wp": np.ascontiguousarray(wp).astype(BF16NP),
        "w1": _pack_dr(g2[:, None] * w1),
        "w2": np.ascontiguousarray(w2).astype(BF16NP),
        "colq": np.ascontiguousarray(
            (QS * (be1 @ wq)).reshape(KC, P).T),
        "colk": np.ascontiguousarray(
            (QS * (be1 @ wk)).reshape(KC, P).T),
        "b1c": np.ascontiguousarray(
            (HS * (b1 + be2 @ w1)).reshape(KC, P).T),
        "bvr": (WS * (be1 @ wv)).astype(BF16NP),
        "bp": np.asarray(inputs["bproj"], dtype=f),
        "b2": np.asarray(inputs["b2"], dtype=f),
    }
    return [{"x": x[b], **common} for b in range(N_CORES)]


def kernel(**inputs) -> np.ndarray:
    if "nc" not in _CACHE:
        _CACHE["nc"] = build_nc()
    nc = _CACHE["nc"]
    in_maps = _prep_inputs(inputs)
    res = run_bass_kernel_spmd(nc, in_maps, list(range(N_CORES)))
    out = np.stack([res.results[b]["out"] for b in range(N_CORES)], axis=0)
    return out.astype(np.float32)


if __name__ == "__main__":
    rng = np.random.default_rng(0)
    demo = {
        "x": rng.standard_normal((B, T, C), dtype=np.float32),
        "Wq": rng.standard_normal((H, C, Dh), dtype=np.float32) * 0.02,
        "Wk": rng.standard_normal((H, C, Dh), dtype=np.float32) * 0.02,
        "Wv": rng.standard_normal((H, C, Dh), dtype=np.float32) * 0.02,
        "Wproj": rng.standard_normal((C, C), dtype=np.float32) * 0.02,
        "bproj": np.zeros(C, np.float32),
        "W1": rng.standard_normal((C, C), dtype=np.float32) * 0.02,
        "b1": np.zeros(C, np.float32),
        "W2": rng.standard_normal((C, C), dtype=np.float32) * 0.02,
        "b2": np.zeros(C, np.float32),
        "g1": np.ones(C, np.float32),
        "beta1": np.zeros(C, np.float32),
        "g2": np.ones(C, np.float32),
        "beta2": np.zeros(C, np.float32),
    }
    y = kernel(**demo)
    print("out", y.shape, y.dtype, float(np.abs(y).max()))


# revision 56
# speedup vs baseline: 1.9416x; 1.0199x over previous
"""Trainium2 Bass kernel for nn_Block_3616362463321 (dense transformer block).

B=8, T=1024, C=1024, H=16, Dh=64. Data-parallel over batch: core b gets x[b].
Weights replicated to all 8 cores; no collectives.

v2 design (vs baseline):
  - All six weight GEMMs (QKV / proj / fc1 / fc2) run fp8e4m3 with
    perf_mode=DoubleRow: K=256 contraction per pass, 0.5 cycles/row.
    Weights are host-quantized (x1024, clip +-240) into an interleaved
    [q, p, islot, m] layout; activations are quantized on the fly into
    "mega" SBUF tiles [128, 8*1024] fp8 whose (k=c-chunk, t) layout serves
    both the DoubleRow moving-operand view [p, 2, t] and the stationary
    view [p, 2, 128].
  - LayerNorm statistics AND normalization happen in natural [t, c] layout
    (per-partition mean/rstd -> one tensor_scalar), with gamma folded into
    the weights host-side and beta folded into per-output bias columns.
    No DRAM stat bounces.  Normalized activations are PE-transposed as fp8
    (1 cycle/row), 4 blocks batched per PSUM bank.
  - proj and fc2 run bf16 (plain matmuls) - fp8 weights there cost too much
    accuracy on the direct output paths; V/attnT/E/y stay bf16 for the same
    reason.  Q/K/V/fc1 keep fp8 DoubleRow.
  - Attention keeps the baseline S^T orientation (fp8 Q/K, bf16 E, bf16 V
    with a fused ones column for softmax denominators), with:
      * S row-tile pairs (K=64 at rows 0/64) writing one 2-bank PSUM pair
        tile -> a single paired exp per (m, tn, i) on ACT,
      * exact causal trims everywhere (bf16/fp8 matmuls have no N>=256
        restriction),
      * causal masking as a post-exp bf16 triangle multiply (2x DVE mode),
      * pa tiles evacuated to SBUF immediately (frees PSUM banks from the
        denominator-bounce latency); sums rows reshaped to [128, 8] via DMA
        so the bit-exact DVE reciprocal runs lane-parallel (a [1,512]
        reciprocal is ~6 cycles/element serial), then a DRAM bounce
        broadcasts 1/sums; the normalize multiplies run on GpSimd.
  - fc2 residual fused via scalar_tensor_tensor; SBUF-only elementwise work
    (x+bias rows, y+b2, normalize mults) offloaded to the idle GpSimd
    engine; x/out rows on the SP DMA queue, weights on Pool/ACT queues.
"""
import sys

sys.path.insert(0, "/opt/trn_rl_repo")

from contextlib import ExitStack, nullcontext

import numpy as np
import ml_dtypes

import concourse.bacc as bacc
import concourse.bass as bass
import concourse.mybir as mybir
import concourse.tile as tile
from concourse.bass_utils import run_bass_kernel_spmd

P = 128
B, T, C, H = 8, 1024, 1024, 16
Dh = C // H            # 64
EPS = 1e-5
NF = 512               # matmul moving free dim (fp32 PSUM bank limit)
KC = C // P            # 8 c-chunks of 128
QC = C // 256          # 4 c-chunks of 256 (DoubleRow)
TJ = T // P            # 8 t-chunks of 128
TN = T // NF           # 2 t-chunks of 512
F32 = mybir.dt.float32
F32R = mybir.dt.float32r
BF16 = mybir.dt.bfloat16
F8 = mybir.dt.float8e4
ALU = mybir.AluOpType
ACTF = mybir.ActivationFunctionType
DR = mybir.MatmulPerfMode.DoubleRow

WS = 1024.0            # host weight upscale (fp8 range use)
QS = 8.0               # Q/K storage scale
SEXP = (Dh ** -0.5) / (QS * QS)   # exp scale absorbing Q/K storage scales
VIS = 1.0              # V ones-column value (bf16 attnT: true scale)
HS = 8.0               # hT storage scale

N_CORES = 8

_CACHE = {}

F8NP = ml_dtypes.float8_e4m3
BF16NP = ml_dtypes.bfloat16


def _bcast_row_ap(handle_ap, parts):
    """AP reading a [N]-shaped DRAM tensor broadcast across `parts` partitions."""
    return bass.AP(
        tensor=handle_ap.tensor,
        offset=handle_ap.offset,
        ap=[[0, parts], *handle_ap.ap],
    )


def build_nc(loop=1, hwloop=0, phases=7):
    nc = bacc.Bacc("TRN2", target_bir_lowering=False, debug=False)

    x_d = nc.dram_tensor("x", [T, C], F32, kind="ExternalInput")
    w_d = {}
    for nm in ("wq", "wk", "wv", "w1"):
        w_d[nm] = nc.dram_tensor(nm, [QC * P, 2 * C], F8, kind="ExternalInput")
    wp_d = nc.dram_tensor("wp", [C, C], BF16, kind="ExternalInput")
    w2_d = nc.dram_tensor("w2", [C, C], BF16, kind="ExternalInput")
    # bias columns pre-shaped host-side to [P, KC] (contiguous DMA; the
    # strided "(k p) -> p k" load costs 1024 4-byte descriptors)
    colq_d = nc.dram_tensor("colq", [P, KC], F32, kind="ExternalInput")
    colk_d = nc.dram_tensor("colk", [P, KC], F32, kind="ExternalInput")
    b1c_d = nc.dram_tensor("b1c", [P, KC], F32, kind="ExternalInput")
    bvr_d = nc.dram_tensor("bvr", [C], BF16, kind="ExternalInput")
    bp_d = nc.dram_tensor("bp", [C], F32, kind="ExternalInput")
    b2_d = nc.dram_tensor("b2", [C], F32, kind="ExternalInput")
    out_d = nc.dram_tensor("out", [T, C], F32, kind="ExternalOutput")

    identb_c = nc.inline_tensor(np.eye(P).astype(BF16NP), name="identb_c")
    # post-exp causal keep-mask for a diagonal [s,t] block: keep where s <= t
    tri_np = (np.arange(P)[:, None] <= np.arange(P)[None, :]).astype(BF16NP)
    tri_c = nc.inline_tensor(tri_np, name="tri_c")

    with tile.TileContext(nc) as tc, ExitStack() as ES:
        singles = ES.enter_context(tc.tile_pool(name="singles", bufs=1))
        dram = ES.enter_context(tc.tile_pool(name="drsc", bufs=1, space="DRAM"))

        identb = singles.tile([P, P], BF16)
        nc.sync.dma_start(out=identb[:], in_=identb_c.ap())
        tri01 = singles.tile([P, P], BF16)
        nc.sync.dma_start(out=tri01[:], in_=tri_c.ap())
        epsc = singles.tile([P, 1], F32)
        nc.vector.memset(epsc[:], EPS)
        onesb = singles.tile([1, P], BF16)
        nc.vector.memset(onesb[:], 1.0)

        # bias columns [P, KC]: element (p, k) = vec[k*P + p]
        cols = {}

        def load_col(nm, hd):
            t_ = singles.tile([P, KC], F32, tag=f"col_{nm}", name=f"col_{nm}")
            nc.gpsimd.dma_start(out=t_[:], in_=hd[:, :])
            cols[nm] = t_

        load_col("q", colq_d)
        load_col("k", colk_d)
        load_col("b1", b1c_d)
        bvr = singles.tile([1, C], BF16)
        nc.gpsimd.dma_start(out=bvr[:], in_=bvr_d.ap()[None, :])
        bpb = singles.tile([P, C], F32)
        nc.gpsimd.dma_start(out=bpb[:], in_=_bcast_row_ap(bp_d.ap(), P))
        b2b = singles.tile([P, C], F32)
        nc.gpsimd.dma_start(out=b2b[:], in_=_bcast_row_ap(b2_d.ap(), P))

        # ---- SBUF arena ----
        arena = ES.enter_context(tc.tile_pool(name="arena", bufs=1))

        def mega(tag):
            return arena.tile([P, KC * T], F8, tag=tag, name=tag)

        # weight tiles: [P, 2, C] fp8 per 256-chunk
        def wtiles(nm, share=None):
            tg = share or nm
            return [arena.tile([P, 2, C], F8, tag=f"{tg}_{q}", name=f"{nm}{q}")
                    for q in range(QC)]

        # ---- PSUM pool: declare SP pair tags first (2 banks each), then PA ----
        psum = ES.enter_context(tc.tile_pool(name="psum", bufs=1, space="PSUM"))
        _sp = [0]
        _pa = [0]

        def sptile(shape, dtype, nm="sp"):
            t = psum.tile(list(shape), dtype, tag=f"SP{_sp[0] % 2}",
                          name=f"{nm}{_sp[0]}")
            _sp[0] += 1
            return t

        def patile(shape=(P, NF), nm="pa"):
            t = psum.tile(list(shape), F32, tag=f"PA{_pa[0] % 4}",
                          name=f"{nm}{_pa[0]}")
            _pa[0] += 1
            return t

        # force tag declaration order: SP0, SP1 as [P, 2, NF] f32 (2 banks each)
        _ = psum.tile([P, 2, NF], F32, tag="SP0", name="spdecl0")
        _ = psum.tile([P, 2, NF], F32, tag="SP1", name="spdecl1")

        # weights on the Pool/ACT queues; x and out rows keep SP (+ACT) free-ish
        _dq = [0]
        _dqe = [nc.gpsimd, nc.scalar]

        def bulk_dma(out, in_):
            eng = _dqe[_dq[0] % len(_dqe)]
            _dq[0] += 1
            eng.dma_start(out=out, in_=in_)

        def row_dma(out, in_):
            nc.sync.dma_start(out=out, in_=in_)

        def ln_pass(src_tiles_or_loader, xn_tag, dst_mega, ph, out_rows_dtype=F8):
            """Natural-layout LN: per 128-row chunk j, bn_stats -> mean/rstd
            columns -> one tensor_scalar into an fp8 row tile -> PE-transpose
            (fp8, 4 blocks per PSUM batch) into dst_mega [(k t)] layout."""
            with ExitStack() as S:
                stp = S.enter_context(tc.tile_pool(name=f"stp{ph}", bufs=4))
                xnp = S.enter_context(tc.tile_pool(name=f"xnp{ph}", bufs=3))
                dv = dst_mega[:].rearrange("p (k t) -> p k t", k=KC)
                for j in range(TJ):
                    xj = src_tiles_or_loader(j)
                    st = stp.tile([P, 2, 6], F32, tag="st")
                    xr2 = xj[:].rearrange("p (g f) -> p g f", f=NF)
                    for g in range(2):
                        nc.vector.bn_stats(out=st[:, g, :], in_=xr2[:, g, :])
                    mv = stp.tile([P, 2], F32, tag="mv")
                    nc.vector.bn_aggr(out=mv[:], in_=st[:])
                    srt = stp.tile([P, 1], F32, tag="srt")
                    nc.scalar.activation(out=srt[:], in_=mv[:, 1:2],
                                         func=ACTF.Sqrt, bias=epsc[:], scale=1.0)
                    rc = stp.tile([P, 1], F32, tag="rc")
                    nc.vector.reciprocal(rc[:], srt[:])
                    xnr = xnp.tile([P, C], BF16, tag="xnr")
                    nc.vector.tensor_scalar(
                        out=xnr[:], in0=xj[:], scalar1=mv[:, 0:1], scalar2=rc[:],
                        op0=ALU.subtract, op1=ALU.mult)
                    for kb in range(2):
                        if kb == 0:
                            pt = sptile([P, 4 * P], BF16, "pt")
                        else:
                            pt = psum.tile([P, 4 * P], BF16,
                                           tag=f"PA{_pa[0] % 4}", name="ptb")
                            _pa[0] += 1
                        for k4 in range(4):
                            k = kb * 4 + k4
                            nc.tensor.transpose(pt[:, k4 * P:(k4 + 1) * P],
                                                xnr[:, k * P:(k + 1) * P],
                                                identb[:])
                        dst = dv[:, kb * 4:(kb + 1) * 4, j * P:(j + 1) * P]
                        src = pt[:].rearrange("p (a b) -> p a b", a=4)
                        if kb == 0:
                            nc.vector.tensor_copy(out=dst, in_=src)
                        else:
                            nc.scalar.activation(out=dst, in_=src,
                                                 func=ACTF.Copy)

        xrp = ES.enter_context(tc.tile_pool(name="xrp", bufs=3))

        def load_x(j):
            xj = xrp.tile([P, C], F32, tag="xrow")
            row_dma(out=xj[:], in_=x_d[j * P:(j + 1) * P, :])
            return xj

        with (tc.For_i(0, hwloop, 1) if hwloop else nullcontext()):
            for _it in range(loop):
                # ---------- weights: issue all DMAs up front (prefetch) ----------
                wq_sb = wtiles("wq")
                wk_sb = wtiles("wk")
                wv_sb = wtiles("wv")
                # w1/w2 reuse wv/wq slots (dead after QKV); DMAs self-order
                w1_sb = wtiles("w1", share="wv")
                w2_sb = [arena.tile([P, C], BF16,
                                    tag=(f"wq_{k}" if k < QC else f"wk_{k - QC}"),
                                    name=f"w2{k}") for k in range(KC)]
                for q in range(QC):
                    for nm, tl in (("wq", wq_sb), ("wk", wk_sb), ("wv", wv_sb)):
                        bulk_dma(out=tl[q][:], in_=w_d[nm][q * P:(q + 1) * P, :])
                # proj weights: plain bf16 [P, C] per c'-chunk
                wp_sb = [arena.tile([P, C], BF16, tag=f"wp_{k}", name=f"wp{k}")
                         for k in range(KC)]
                for k in range(KC):
                    bulk_dma(out=wp_sb[k][:], in_=wp_d[k * P:(k + 1) * P, :])

                # ---------- Phase 0: LN1 (stats + normalize + transpose) ----------
                xn8 = mega("XN1")
                ln_pass(load_x, "xn", xn8, 0)
                xnv = xn8[:].rearrange("p (k t) -> p k t", k=KC)
                # dummy exp: pulls the exp ACT-table load off the
                # attention-start critical path into the QKV stretch
                scr = singles.tile([1, 1], F32, tag="dscr", name="dscr")
                nc.scalar.activation(out=scr[:], in_=epsc[0:1, 0:1],
                                     func=ACTF.Exp, scale=0.0)

                if phases >= 2:
                    # ---------------- Phase 1: QKV (fp8 DoubleRow) ----------------
                    QT = [arena.tile([P, T], F8, tag=f"QT_{m}", name=f"QT{m}")
                          for m in range(KC)]
                    KTt = [arena.tile([P, T], F8, tag=f"KT_{m}", name=f"KT{m}")
                           for m in range(KC)]
                    for tn in range(TN):
                        tsl = slice(tn * NF, (tn + 1) * NF)
                        for m in range(KC):
                            pq = patile(nm="pq")
                            for q in range(QC):
                                nc.tensor.matmul(
                                    pq[:], lhsT=wq_sb[q][:, :, m * P:(m + 1) * P],
                                    rhs=xnv[:, 2 * q:2 * q + 2, tsl],
                                    start=(q == 0), stop=(q == QC - 1),
                                    perf_mode=DR)
                            nc.vector.tensor_scalar(
                                out=QT[m][:, tsl], in0=pq[:], scalar1=QS / WS,
                                scalar2=cols["q"][:, m:m + 1],
                                op0=ALU.mult, op1=ALU.add)
                            pk = patile(nm="pk")
                            for q in range(QC):
                                nc.tensor.matmul(
                                    pk[:], lhsT=wk_sb[q][:, :, m * P:(m + 1) * P],
                                    rhs=xnv[:, 2 * q:2 * q + 2, tsl],
                                    start=(q == 0), stop=(q == QC - 1),
                                    perf_mode=DR)
                            nc.vector.tensor_scalar(
                                out=KTt[m][:, tsl], in0=pk[:], scalar1=QS / WS,
                                scalar2=cols["k"][:, m:m + 1],
                                op0=ALU.mult, op1=ALU.add)

                    V = [arena.tile([P, H, Dh + 1], BF16, tag=f"V_{j}",
                                    name=f"V{j}") for j in range(TJ)]
                    for j in range(TJ):
                        nc.gpsimd.memset(V[j][:, :, Dh:Dh + 1], VIS)
                        for hn in range(TN):
                            hsl = slice(hn * NF, (hn + 1) * NF)
                            pv = patile(nm="pv")
                            nc.tensor.matmul(pv[:], lhsT=onesb[0:1, :],
                                             rhs=bvr[0:1, hsl],
                                             start=True, stop=False)
                            for q in range(QC):
                                nc.tensor.matmul(
                                    pv[:], lhsT=xnv[:, 2 * q:2 * q + 2,
                                                    j * P:(j + 1) * P],
                                    rhs=wv_sb[q][:, :, hsl],
                                    start=False, stop=(q == QC - 1),
                                    perf_mode=DR)
                            nc.scalar.activation(
                                out=V[j][:, hn * 8:(hn + 1) * 8, 0:Dh],
                                in_=pv[:].rearrange("p (h d) -> p h d", d=Dh),
                                func=ACTF.Identity, scale=1.0 / WS)
                    # late weights into the now-free wv/wq/wk slots
                    for q in range(QC):
                        bulk_dma(out=w1_sb[q][:],
                                 in_=w_d["w1"][q * P:(q + 1) * P, :])
                    for k in range(KC):
                        bulk_dma(out=w2_sb[k][:],
                                 in_=w2_d[k * P:(k + 1) * P, :])

                # proj residual rows (x + bproj) prefetched before attention
                # so proj can start the moment its attnT half is ready
                xrb = [arena.tile([P, C], BF16, tag=f"XRB_{j}", name=f"xrb{j}")
                       for j in range(TJ)]
                for j in range(TJ):
                    xj2 = load_x(j)
                    nc.gpsimd.tensor_tensor(xrb[j][:], xj2[:], bpb[:], ALU.add)

                attnT = arena.tile([P, KC * T], BF16, tag="ATT", name="attnT")
                atv = attnT[:].rearrange("p (m t) -> p m t", m=KC)
                if phases >= 3:
                    # ---------------- Phase 2: attention ----------------
                    with ExitStack() as S:
                        ep = S.enter_context(tc.tile_pool(name="ep", bufs=3))
                        rp = S.enter_context(tc.tile_pool(name="rp", bufs=4))
                        bp_ = S.enter_context(tc.tile_pool(name="bp", bufs=3))
                        tp1 = S.enter_context(tc.tile_pool(name="tp1", bufs=3))
                        for tn in range(TN):
                            tsl = slice(tn * NF, (tn + 1) * NF)
                            i_hi = 4 * (tn + 1)
                            for m in range(KC):
                                h0, h1 = 2 * m, 2 * m + 1
                                pa0 = patile((Dh + 1, NF), "pa0")
                                pa1 = patile((Dh + 1, NF), "pa1")
                                for i in range(i_hi):
                                    diag = i - 4 * tn
                                    d0 = max(diag, 0) * P
                                    esl = slice(d0, NF)
                                    qsl = slice(tn * NF + d0, (tn + 1) * NF)
                                    ssl = slice(i * P, (i + 1) * P)
                                    sp2 = sptile([P, 2, NF], F32, "s")
                                    nc.tensor.matmul(
                                        sp2[:, 0, esl], lhsT=KTt[m][0:64, ssl],
                                        rhs=QT[m][0:64, qsl],
                                        start=True, stop=True)
                                    nc.tensor.matmul(
                                        sp2[:, 1, esl], lhsT=KTt[m][64:128, ssl],
                                        rhs=QT[m][64:128, qsl],
                                        start=True, stop=True)
                                    Et = ep.tile([P, 2, NF], BF16, tag="E")
                                    nc.scalar.activation(
                                        out=Et[:, :, esl], in_=sp2[:, :, esl],
                                        func=ACTF.Exp, scale=SEXP)
                                    if diag >= 0:
                                        dsl = slice(d0, d0 + P)
                                        tri_b = bass.AP(
                                            tensor=tri01[:].tensor,
                                            offset=tri01[:].offset,
                                            ap=[tri01[:].ap[0], [0, 2],
                                                *tri01[:].ap[1:]])
                                        nc.vector.tensor_tensor(
                                            Et[:, :, dsl], Et[:, :, dsl],
                                            tri_b, ALU.mult)
                                    nc.tensor.matmul(
                                        pa0[:, esl], lhsT=V[i][:, h0, :],
                                        rhs=Et[:, 0, esl],
                                        start=(i == 0), stop=(i == i_hi - 1))
                                    nc.tensor.matmul(
                                        pa1[:, esl], lhsT=V[i][:, h1, :],
                                        rhs=Et[:, 1, esl],
                                        start=(i == 0), stop=(i == i_hi - 1))
                                # evacuate pa to SBUF immediately (frees the
                                # PSUM banks from the denominator-bounce
                                # latency), then: reshape sums to [128, 8] via
                                # DMA so the bit-exact reciprocal runs wide,
                                # bounce through DRAM for the broadcast, and
                                # normalize on the idle Pool engine.
                                av0 = rp.tile([Dh + 1, NF], BF16, tag="av0")
                                av1 = rp.tile([Dh + 1, NF], BF16, tag="av1")
                                nc.vector.tensor_copy(out=av0[:], in_=pa0[:])
                                nc.vector.tensor_copy(out=av1[:], in_=pa1[:])
                                s2b = rp.tile([P, 8], BF16, tag="s2b")
                                nc.gpsimd.dma_start(
                                    out=s2b[:, 0:4],
                                    in_=av0[Dh:Dh + 1, :])
                                nc.gpsimd.dma_start(
                                    out=s2b[:, 4:8],
                                    in_=av1[Dh:Dh + 1, :])
                                s2 = rp.tile([P, 8], F32, tag="s2")
                                nc.vector.reciprocal(s2[:], s2b[:])
                                drr = dram.tile([2 * NF], F32, tag="rsums")
                                nc.gpsimd.dma_start(
                                    out=drr[0:NF].rearrange("(p i) -> p i", i=4),
                                    in_=s2[:, 0:4])
                                nc.gpsimd.dma_start(
                                    out=drr[NF:2 * NF].rearrange(
                                        "(p i) -> p i", i=4),
                                    in_=s2[:, 4:8])
                                bct = bp_.tile([Dh, 2, NF], F32, tag="bct")
                                nc.sync.dma_start(
                                    out=bct[:, 0, :],
                                    in_=drr[0:NF][None, :].to_broadcast([Dh, NF]))
                                nc.sync.dma_start(
                                    out=bct[:, 1, :],
                                    in_=drr[NF:2 * NF][None, :].to_broadcast(
                                        [Dh, NF]))
                                nc.gpsimd.tensor_tensor(
                                    atv[0:Dh, m, tsl], av0[0:Dh, :],
                                    bct[:, 0, :], ALU.mult)
                                tmp1 = tp1.tile([Dh, NF], BF16, tag="t1")
                                nc.gpsimd.tensor_tensor(
                                    tmp1[:], av1[0:Dh, :], bct[:, 1, :], ALU.mult)
                                nc.gpsimd.dma_start(
                                    out=atv[Dh:2 * Dh, m, tsl], in_=tmp1[:])

                y_n = [arena.tile([P, C], BF16, tag=f"Y_{j}", name=f"y{j}")
                       for j in range(TJ)]
                if phases >= 4:
                    # ---------- Phase 3: proj + residual -> y (bf16, normal) -----
                    for j in range(TJ):
                        for nn in range(TN):
                            csl = slice(nn * NF, (nn + 1) * NF)
                            pp = patile(nm="pp")
                            for k in range(KC):
                                nc.tensor.matmul(
                                    pp[:], lhsT=atv[:, k, j * P:(j + 1) * P],
                                    rhs=wp_sb[k][:, csl],
                                    start=(k == 0), stop=(k == KC - 1))
                            nc.vector.tensor_tensor(
                                y_n[j][:, csl], pp[:], xrb[j][:, csl], ALU.add)

                if phases >= 5:
                    # ---------------- Phase 4: LN2 ----------------
                    xn28 = mega("XN2")
                    ln_pass(lambda j: y_n[j], "xn2", xn28, 1)
                    xn2v = xn28[:].rearrange("p (k t) -> p k t", k=KC)

                hT = arena.tile([P, KC * T], BF16, tag="HT", name="hT")
                htv = hT[:].rearrange("p (m t) -> p m t", m=KC)
                if phases >= 6:
                    # ---------------- Phase 5: MLP fc1 + relu ----------------
                    for tn in range(TN):
                        tsl = slice(tn * NF, (tn + 1) * NF)
                        for m in range(KC):
                            ph = patile(nm="ph")
                            for q in range(QC):
                                nc.tensor.matmul(
                                    ph[:], lhsT=w1_sb[q][:, :, m * P:(m + 1) * P],
                                    rhs=xn2v[:, 2 * q:2 * q + 2, tsl],
                                    start=(q == 0), stop=(q == QC - 1),
                                    perf_mode=DR)
                            nc.scalar.activation(
                                out=htv[:, m, tsl], in_=ph[:], func=ACTF.Relu,
                                bias=cols["b1"][:, m:m + 1], scale=HS / WS)

                if phases >= 7:
                    # ---------- Phase 6: MLP fc2 + residual -> out ----------
                    with ExitStack() as S:
                        otp = S.enter_context(tc.tile_pool(name="otp", bufs=3))
                        y2p = S.enter_context(tc.tile_pool(name="y2p", bufs=2))
                        for j in range(TJ):
                            y2 = y2p.tile([P, C], BF16, tag="y2")
                            nc.gpsimd.tensor_tensor(y2[:], y_n[j][:], b2b[:],
                                                    ALU.add)
                            for nn in range(TN):
                                csl = slice(nn * NF, (nn + 1) * NF)
                                po = patile(nm="po")
                                for k in range(KC):
                                    nc.tensor.matmul(
                                        po[:], lhsT=htv[:, k, j * P:(j + 1) * P],
                                        rhs=w2_sb[k][:, csl],
                                        start=(k == 0), stop=(k == KC - 1))
                                ot = otp.tile([P, NF], F32, tag="ot")
                                nc.vector.scalar_tensor_tensor(
                                    out=ot[:], in0=po[:], scalar=1.0 / HS,
                                    in1=y2[:, csl], op0=ALU.mult, op1=ALU.add)
                                bulk_dma(out=out_d[j * P:(j + 1) * P, csl],
                                         in_=ot[:])

    nc.compile()
    return nc


def _f8(a):
    return np.clip(a, -240.0, 240.0).astype(F8NP)


def _pack_dr(w_eff):
    """[C, M] effective weight -> [QC*P, 2*M] fp8 DoubleRow layout
    (row q*128+p, col i*M+m  <-  w_eff[q*256 + i*128 + p, m] * WS)."""
    M = w_eff.shape[1]
    w = (w_eff * WS).reshape(QC, 2, P, M).transpose(0, 2, 1, 3).reshape(
        QC * P, 2 * M)
    return _f8(np.ascontiguousarray(w))


def _prep_inputs(inputs):
    """Host-side weight repacking/quantization; returns per-core in_maps."""
    f = np.float32
    x = np.ascontiguousarray(np.asarray(inputs["x"], dtype=f))        # [B, T, C]
    g1 = np.asarray(inputs["g1"], dtype=f)
    be1 = np.asarray(inputs["beta1"], dtype=f)
    g2 = np.asarray(inputs["g2"], dtype=f)
    be2 = np.asarray(inputs["beta2"], dtype=f)

    wq = np.asarray(inputs["Wq"], dtype=f).transpose(1, 0, 2).reshape(C, C)
    wk = np.asarray(inputs["Wk"], dtype=f).transpose(1, 0, 2).reshape(C, C)
    wv = np.asarray(inputs["Wv"], dtype=f).transpose(1, 0, 2).reshape(C, C)
    wp = np.asarray(inputs["Wproj"], dtype=f)
    w1 = np.asarray(inputs["W1"], dtype=f)
    w2 = np.asarray(inputs["W2"], dtype=f)
    b1 = np.asarray(inputs["b1"], dtype=f)

    common = {
        "wq": _pack_dr(g1[:, None] * wq),
        "wk": _pack_dr(g1[:, None] * wk),
        "wv": _pack_dr(g1[:, None] * wv),
        "wp": np.ascontiguousarray(wp).astype(BF16NP),
        "w1": _pack_dr(g2[:, None] * w1),
        "w2": np.ascontiguousarray(w2).astype(BF16NP),
        "colq": np.ascontiguousarray(
            (QS * (be1 @ wq)).reshape(KC, P).T),
        "colk": np.ascontiguousarray(
            (QS * (be1 @ wk)).reshape(KC, P).T),
        "b1c": np.ascontiguousarray(
            (HS * (b1 + be2 @ w1)).reshape(KC, P).T),
        "bvr": (WS * (be1 @ wv)).astype(BF16NP),
        "bp": np.asarray(inputs["bproj"], dtype=f),
        "b2": np.asarray(inputs["b2"], dtype=f),
    }
    return [{"x": x[b], **common} for b in range(N_CORES)]


def kernel(**inputs) -> np.ndarray:
    if "nc" not in _CACHE:
        _CACHE["nc"] = build_nc()
    nc = _CACHE["nc"]
    in_maps = _prep_inputs(inputs)
    res = run_bass_kernel_spmd(nc, in_maps, list(range(N_CORES)))
    out = np.stack([res.results[b]["out"] for b in range(N_CORES)], axis=0)
    return out.astype(np.float32)


if __name__ == "__main__":
    rng = np.random.default_rng(0)
    demo = {
        "x": rng.standard_normal((B, T, C), dtype=np.float32),
        "Wq": rng.standard_normal((H, C, Dh), dtype=np.float32) * 0.02,
        "Wk": rng.standard_normal((H, C, Dh), dtype=np.float32) * 0.02,
        "Wv": rng.standard_normal((H, C, Dh), dtype=np.float32) * 0.02,
        "Wproj": rng.standard_normal((C, C), dtype=np.float32) * 0.02,
        "bproj": np.zeros(C, np.float32),
        "W1": rng.standard_normal((C, C), dtype=np.float32) * 0.02,
        "b1": np.zeros(C, np.float32),
        "W2": rng.standard_normal((C, C), dtype=np.float32) * 0.02,
        "b2": np.zeros(C, np.float32),
        "g1": np.ones(C, np.float32),
        "beta1": np.zeros(C, np.float32),
        "g2": np.ones(C, np.float32),
        "beta2": np.zeros(C, np.float32),
    }
    y = kernel(**demo)
    print("out", y.shape, y.dtype, float(np.abs(y).max()))
